# revision 1
# baseline (speedup 1.0000x reference)
"""Fused pre-LN + QKV + attention + post-LN + residual kernel for TRN2.

Problem (nn_Attention_86517821210894):
    B=2, N=4096, C=512, H=8, D=64
    xn  = LN(x) ; qkv = xn @ w_qkv + b ; per-(b,h) softmax attention
    val = LN(attn_out) ; out = xn + val

Sharding (8 cores, zero collectives):
    core c -> batch b = c // 4, query-row block r = c % 4 (1024 rows).
    Each core receives x[b] ROTATED so its query block is rows 0:1024
    (softmax and the value sum are permutation-invariant over keys, so
    rotating the key order changes nothing), builds K/V for all 4096
    keys, and produces out[b, r*1024:(r+1)*1024].  The K/V qkv matmul is
    recomputed by the 4 cores of a batch; this trades ~30% extra PE work
    for zero inter-core communication.

Device pipeline per core (emission order is tuned so Tile's scheduler
keeps ScalarE's exp stream — the critical resource at ~290us — fed from
~20us onward):
    1. pre-LN: bn_stats/bn_aggr on VectorE, batched sqrt (4 row-tiles per
       Sqrt) and the normalize itself on ScalarE (Identity with per-
       partition scale/bias), PE-transpose into xnT [C, N] bf16
    2. qkv from xnT, chasing the transposes column-group by column-group:
       v [N, 8, 65] (ones column -> AV also computes the softmax
       denominator), kT [2h*64, N] per head pair, qT [2h*64, 1024]
    3. scoresT[k, q] = kT-slice.T @ qT-slice, two heads packed in the PE
       array (partition row groups 0-63/64-127), grouped 3 slices per
       PSUM tile so each ScalarE exp covers 1536 elem/partition.
       Max-subtraction is skipped: scores*0.125 has |z| < ~2 for LN'd
       inputs so exp is safely in range; softmax is shift-invariant so
       the result matches the reference.
    4. AV: valT_aug[65, q] accumulated over key chunks in PSUM
    5. per qb half: transpose valT row-major, divide by the denominator
       column, post-LN (batched Sqrt), add xn residual, DMA out.
"""

import sys

sys.path.insert(0, "/opt/trn_rl_repo")

import numpy as np

B, N, C, H = 2, 4096, 512, 8
D = C // H
QR = N // 4  # query rows per core
EPS = 1e-5
SCALE = float(D) ** -0.5

_CACHE = {}


def _build(flags):
    (use_g_pre, use_beta_pre, use_g_post, use_beta_post, use_b_q, use_b_v) = flags

    import concourse.bacc as bacc
    import concourse.bass as bass
    import concourse.tile as tile
    from concourse import mybir
    from concourse.masks import make_identity

    f32 = mybir.dt.float32
    bf16 = mybir.dt.bfloat16
    AF = mybir.ActivationFunctionType
    ALU = mybir.AluOpType

    nc = bacc.Bacc(
        "TRN2", target_bir_lowering=False, debug=False, enable_asserts=False
    )

    xb = nc.dram_tensor("xb", [N, C], f32, kind="ExternalInput").ap()
    w = nc.dram_tensor("w_qkv", [C, 3 * C], bf16, kind="ExternalInput").ap()
    bqkv = nc.dram_tensor("b_qkv", [3 * C], f32, kind="ExternalInput").ap()
    g_pre = nc.dram_tensor("g_pre", [C], f32, kind="ExternalInput").ap()
    beta_pre = nc.dram_tensor("beta_pre", [C], f32, kind="ExternalInput").ap()
    g_post = nc.dram_tensor("g_post", [C], f32, kind="ExternalInput").ap()
    beta_post = nc.dram_tensor("beta_post", [C], f32, kind="ExternalInput").ap()
    out = nc.dram_tensor("out", [QR, C], f32, kind="ExternalOutput").ap()

    NT = N // 128  # 32 row tiles of x[b]
    QT = QR // 128  # 8 row tiles of the query block
    CCH = C // 128  # 4 contraction chunks
    KC = N // 128  # 32 key chunks
    NPAIR = H // 2
    NQB = QR // 512  # 2 query blocks of 512

    def bcast(vec_ap, p):
        return bass.AP(
            tensor=vec_ap.tensor, offset=vec_ap.offset, ap=[[0, p], *vec_ap.ap]
        )

    with tile.TileContext(nc) as tc:
        with (
            tc.tile_pool(name="consts", bufs=1) as consts,
            tc.tile_pool(name="ln_in", bufs=4) as ln_in,
            tc.tile_pool(name="stats", bufs=8) as stats,
            tc.tile_pool(name="xnrow", bufs=1) as xnrow_pool,
            tc.tile_pool(name="xnT", bufs=1) as xnT_pool,
            tc.tile_pool(name="vsb", bufs=1) as v_pool,
            tc.tile_pool(name="kT", bufs=4) as kT_pool,
            tc.tile_pool(name="qT", bufs=4) as qT_pool,
            tc.tile_pool(name="expT", bufs=3) as expT_pool,
            tc.tile_pool(name="valT", bufs=2) as valT_pool,
            tc.tile_pool(name="valasm", bufs=1) as val_pool,
            tc.tile_pool(name="outp", bufs=4) as out_pool,
            tc.tile_pool(name="ps3", bufs=2, space="PSUM") as ps3,
            tc.tile_pool(name="pav", bufs=2, space="PSUM") as psum_av,
        ):
            # ---- warmup burst: bring the PE HAM to K=8/8 immediately ----
            dummy = consts.tile([128, 512], bf16)
            nc.gpsimd.memset(dummy, 0.0)
            pw = ps3.tile([128, 3, 512], f32, tag="ps3")
            for _ in range(16):
                nc.tensor.matmul(pw[:, 0, :], dummy[:, 0:128], dummy)
            del pw

            # ---- constants ----
            ident = consts.tile([128, 128], f32)
            make_identity(nc, ident)
            ident_bf = consts.tile([128, 128], bf16)
            make_identity(nc, ident_bf)
            eps_t = consts.tile([128, 1], f32)
            nc.vector.memset(eps_t, EPS)
            seed_b = consts.tile([128, 1], f32)
            nc.vector.memset(seed_b, 0.5 * 0.6931471805599453 * 127.0)

            w_sb = consts.tile([128, CCH, 3 * C], bf16)
            nc.sync.dma_start(out=w_sb, in_=w.rearrange("(cc p) m -> p cc m", p=128))

            g_pre_t = beta_pre_t = g_post_t = beta_post_t = None
            if use_g_pre:
                g_pre_t = consts.tile([128, C], f32)
                nc.sync.dma_start(out=g_pre_t, in_=bcast(g_pre, 128))
            if use_beta_pre:
                beta_pre_t = consts.tile([128, C], f32)
                nc.sync.dma_start(out=beta_pre_t, in_=bcast(beta_pre, 128))
            if use_g_post:
                g_post_t = consts.tile([128, C], f32)
                nc.sync.dma_start(out=g_post_t, in_=bcast(g_post, 128))
            if use_beta_post:
                beta_post_t = consts.tile([128, C], f32)
                nc.sync.dma_start(out=beta_post_t, in_=bcast(beta_post, 128))
            bq_t = None
            if use_b_q:
                bq_t = consts.tile([128, CCH, 1], f32)
                nc.sync.dma_start(
                    out=bq_t, in_=bqkv[0:C].rearrange("(cc p) -> p cc 1", p=128)
                )
            bv_t = None
            if use_b_v:
                bv_t = consts.tile([128, C], f32)
                nc.sync.dma_start(out=bv_t, in_=bcast(bqkv[2 * C : 3 * C], 128))

            # ---- persistent tensors ----
            xn_rows = xnrow_pool.tile([128, QT, C], f32)
            xnT = xnT_pool.tile([128, CCH, N], bf16)
            v_sb = v_pool.tile([128, KC, H, D + 1], bf16)
            val_asm = val_pool.tile([128, QT, H, D + 1], f32)

            nc.vector.memset(v_sb[:, :, :, D : D + 1], 1.0)

            def rsqrt_into(dst, a4, w, tag):
                """dst = 1/sqrt(a4), a4 > 0, [128, w] f32.

                Bit-trick log2 seed evaluated through ScalarE Exp (which is
                already the resident ACT table for the attention softmax, so
                no table reload) + 3 Newton iterations on VectorE."""
                ai = a4.bitcast(mybir.dt.int32)
                fi = stats.tile([128, w], f32, tag=tag + "_f")
                nc.vector.tensor_copy(out=fi, in_=ai)
                # exp(-0.5*ln2*(i/2^23 - 127)) ~= 1/sqrt(a), rel err <= 3%
                nc.scalar.activation(
                    out=dst,
                    in_=fi,
                    func=AF.Exp,
                    scale=-0.5 * 0.6931471805599453 / 8388608.0,
                    bias=seed_b,
                )
                for _ in range(2):
                    t = stats.tile([128, w], f32, tag=tag + "_t")
                    nc.vector.tensor_mul(out=t, in0=dst, in1=dst)
                    nc.vector.tensor_mul(out=t, in0=t, in1=a4)
                    nc.vector.tensor_scalar(
                        out=t,
                        in0=t,
                        scalar1=-0.5,
                        scalar2=1.5,
                        op0=ALU.mult,
                        op1=ALU.add,
                    )
                    nc.vector.tensor_mul(out=dst, in0=dst, in1=t)

            def transpose_into(dstT, src, col0):
                # src is bf16: single-pass PE transpose (fp32 needs LOW/HIGH
                # double-pumping and costs ~4x here)
                ps = ps3.tile([128, 3, 1024], bf16, tag="ps3")
                pview = ps[:, 0, 0:512].rearrange("p (c n) -> p c n", n=128)
                for cc in range(CCH):
                    nc.tensor.transpose(
                        pview[:, cc, :], src[:, cc * 128 : (cc + 1) * 128], ident_bf
                    )
                nc.vector.tensor_copy(out=dstT[:, :, col0 : col0 + 128], in_=pview)

            def produce_v(kc):
                pv = ps3.tile([128, 3, 512], f32, tag="ps3")
                for cc in range(CCH):
                    nc.tensor.matmul(
                        pv[:, 0, :],
                        xnT[:, cc, kc * 128 : (kc + 1) * 128],
                        w_sb[:, cc, 2 * C : 3 * C],
                        start=(cc == 0),
                        stop=(cc == CCH - 1),
                    )
                src = pv[:, 0, :].rearrange("p (h d) -> p h d", d=D)
                dst = v_sb[:, kc, :, 0:D]
                if use_b_v:
                    nc.vector.tensor_add(
                        out=dst, in0=src, in1=bv_t.rearrange("p (h d) -> p h d", d=D)
                    )
                else:
                    nc.scalar.copy(out=dst, in_=src)

            def produce_kT(pair, kT, rc, eng=None):
                pk = ps3.tile([128, 3, 512], f32, tag="ps3")
                for cc in range(CCH):
                    nc.tensor.matmul(
                        pk[:, 0, :],
                        w_sb[:, cc, C + pair * 128 : C + (pair + 1) * 128],
                        xnT[:, cc, rc * 512 : (rc + 1) * 512],
                        start=(cc == 0),
                        stop=(cc == CCH - 1),
                    )
                (eng or nc.vector).tensor_copy(
                    out=kT[:, rc * 512 : (rc + 1) * 512], in_=pk[:, 0, :]
                )

            def produce_qT(pair, qT, rc, eng=None):
                pq = ps3.tile([128, 3, 512], f32, tag="ps3")
                for cc in range(CCH):
                    nc.tensor.matmul(
                        pq[:, 0, :],
                        w_sb[:, cc, pair * 128 : (pair + 1) * 128],
                        xnT[:, cc, rc * 512 : (rc + 1) * 512],
                        start=(cc == 0),
                        stop=(cc == CCH - 1),
                    )
                if use_b_q:
                    (eng or nc.vector).tensor_scalar_add(
                        out=qT[:, rc * 512 : (rc + 1) * 512],
                        in0=pq[:, 0, :],
                        scalar1=bq_t[:, pair, :],
                    )
                else:
                    (eng or nc.vector).tensor_copy(
                        out=qT[:, rc * 512 : (rc + 1) * 512], in_=pq[:, 0, :]
                    )

            # ---- attention slice machinery ----
            class AttState:
                def __init__(self):
                    self.group = None
                    self.pos = 0
                    self.pending = []
                    self.exp_of = {}
                    self.pavs = None
                    self.av_next = 0

            def flush(st):
                ex = expT_pool.tile([128, 3, 512], bf16, tag="expT")
                nc.scalar.activation(
                    out=ex[:, 0 : st.pos, :],
                    in_=st.group[:, 0 : st.pos, :],
                    func=AF.Exp,
                    scale=SCALE,
                )
                for key, p in st.pending:
                    st.exp_of[key] = (ex, p)
                st.pending.clear()
                st.group = None
                st.pos = 0

            def av_drain(st, pair):
                # emit AV matmuls for every kc whose exp has been flushed;
                # keeps AV interleaved at kc granularity right behind the
                # exp stream so the PE never runs a long AV-only block
                if st.pavs is None:
                    pav_lo = psum_av.tile([128, 512], f32, tag="pav")
                    pav_hi = psum_av.tile([128, 512], f32, tag="pav")
                    st.pavs = (pav_lo, pav_hi)
                while st.av_next < KC and (st.av_next, 1) in st.exp_of:
                    kc = st.av_next
                    for h_idx in range(2):
                        ex, p = st.exp_of.pop((kc, h_idx))
                        nc.tensor.matmul(
                            st.pavs[h_idx][0 : D + 1, :],
                            v_sb[:, kc, 2 * pair + h_idx, :],
                            ex[:, p, :],
                            start=(kc == 0),
                            stop=(kc == KC - 1),
                        )
                    st.av_next += 1

            def scores_chunk(st, pair, qb, kcs, kT, qT, flush_end=True):
                for kc in kcs:
                    for h_idx in range(2):
                        if st.group is None:
                            st.group = ps3.tile([128, 3, 512], f32, tag="ps3")
                            st.pos = 0
                        base = h_idx * 64
                        nc.tensor.matmul(
                            st.group[:, st.pos, :],
                            kT[base : base + 64, kc * 128 : (kc + 1) * 128],
                            qT[base : base + 64, qb * 512 : (qb + 1) * 512],
                        )
                        st.pending.append(((kc, h_idx), st.pos))
                        st.pos += 1
                        if st.pos == 3:
                            flush(st)
                            av_drain(st, pair)
                # in the prefix, never leave a group holding a PSUM slot
                # across other ps3 users; in the bulk streams the scores
                # themselves are the only ps3 users so a group may stay open
                if flush_end and st.group is not None:
                    flush(st)
                    av_drain(st, pair)

            def av_finish(st, pair, qb):
                if st.group is not None:
                    flush(st)
                av_drain(st, pair)
                assert st.av_next == KC
                for h_idx in range(2):
                    h = 2 * pair + h_idx
                    vt = valT_pool.tile([D + 1, 512], f32, tag="valT")
                    nc.vector.tensor_copy(out=vt, in_=st.pavs[h_idx][0 : D + 1, :])
                    for j in range(4):
                        pt = ps3.tile([128, 3, 512], f32, tag="ps3")
                        nc.tensor.transpose(
                            pt[:, 0, 0 : D + 1],
                            vt[:, j * 128 : (j + 1) * 128],
                            ident[0 : D + 1, 0 : D + 1],
                        )
                        nc.vector.tensor_copy(
                            out=val_asm[:, qb * 4 + j, h, :],
                            in_=pt[:, 0, 0 : D + 1],
                        )
                st.pavs = None

            # ---- phase 5 per qb half ----
            def phase5(qtiles):
                ots = []
                m4 = stats.tile([128, 4], f32, tag="m4b")
                a4b = stats.tile([128, 4], f32, tag="a4b")
                r4 = stats.tile([128, 4], f32, tag="r4b")
                for jj, qtile in enumerate(qtiles):
                    va = val_asm[:, qtile]
                    ot = out_pool.tile([128, C], f32, tag="ot")
                    for h in range(H):
                        rs = stats.tile([128, 1], f32, tag="rs")
                        nc.vector.reciprocal(out=rs, in_=va[:, h, D : D + 1])
                        nc.vector.tensor_scalar_mul(
                            out=ot[:, h * D : (h + 1) * D],
                            in0=va[:, h, 0:D],
                            scalar1=rs,
                        )
                    if use_b_v:
                        nc.vector.tensor_add(out=ot, in0=ot, in1=bv_t)
                    st6 = stats.tile([128, 6], f32, tag="bn6")
                    nc.vector.bn_stats(out=st6, in_=ot)
                    mv = stats.tile([128, 2], f32, tag="mv")
                    nc.vector.bn_aggr(out=mv, in_=st6)
                    nc.vector.tensor_copy(out=m4[:, jj : jj + 1], in_=mv[:, 0:1])
                    nc.vector.tensor_copy(out=a4b[:, jj : jj + 1], in_=mv[:, 1:2])
                    ots.append(ot)
                nc.vector.tensor_scalar_add(out=a4b, in0=a4b, scalar1=EPS)
                rsqrt_into(r4, a4b, 4, "p5")
                for jj, qtile in enumerate(qtiles):
                    ot = ots[jj]
                    nc.vector.tensor_scalar(
                        out=ot,
                        in0=ot,
                        scalar1=m4[:, jj : jj + 1],
                        scalar2=r4[:, jj : jj + 1],
                        op0=ALU.subtract,
                        op1=ALU.mult,
                    )
                    if use_g_post:
                        nc.vector.tensor_mul(out=ot, in0=ot, in1=g_post_t)
                    if use_beta_post:
                        nc.vector.tensor_add(out=ot, in0=ot, in1=beta_post_t)
                    nc.vector.tensor_add(out=ot, in0=ot, in1=xn_rows[:, qtile, :])
                    nc.sync.dma_start(
                        out=out[qtile * 128 : (qtile + 1) * 128, :], in_=ot
                    )

            # ================= emission =================
            kTs = [
                kT_pool.tile([128, N], bf16, tag="kT", name=f"kT{p}")
                for p in range(NPAIR)
            ]
            qTs = [
                qT_pool.tile([128, QR], bf16, tag="qT", name=f"qT{p}")
                for p in range(NPAIR)
            ]
            states = [[AttState() for _ in range(NQB)] for _ in range(NPAIR)]

            # phase 1+2 fused: LN stats on DVE, batched rstd on ACT (ln+exp,
            # same table set as the attention exp), normalize on GpSimd,
            # transposes chased by v/kT0/qT0, and pair-0 qb-0 scores+exp+AV
            # trickling one column-group behind to keep ScalarE fed
            st00 = states[0][0]
            for rc in range(NT // 4):
                xts, mvs = [], []
                for j in range(4):
                    i = 4 * rc + j
                    xt = ln_in.tile([128, C], f32, tag="xt")
                    nc.sync.dma_start(out=xt, in_=xb[i * 128 : (i + 1) * 128, :])
                    st6 = stats.tile([128, 6], f32, tag="bn6")
                    nc.vector.bn_stats(out=st6, in_=xt)
                    mv = stats.tile([128, 2], f32, tag="mv")
                    nc.vector.bn_aggr(out=mv, in_=st6)
                    xts.append(xt)
                    mvs.append(mv)
                a4 = stats.tile([128, 4], f32, tag="a4")
                for j in range(4):
                    nc.vector.tensor_copy(out=a4[:, j : j + 1], in_=mvs[j][:, 1:2])
                nc.vector.tensor_scalar_add(out=a4, in0=a4, scalar1=EPS)
                r4 = stats.tile([128, 4], f32, tag="r4")
                rsqrt_into(r4, a4, 4, "p1")
                for j in range(4):
                    i = 4 * rc + j
                    xbf = ln_in.tile([128, C], bf16, tag="xbf")
                    if i < QT or use_g_pre or use_beta_pre:
                        dst = xn_rows[:, i, :] if i < QT else xts[j]
                        nc.vector.tensor_scalar(
                            out=dst,
                            in0=xts[j],
                            scalar1=mvs[j][:, 0:1],
                            scalar2=r4[:, j : j + 1],
                            op0=ALU.subtract,
                            op1=ALU.mult,
                        )
                        if use_g_pre:
                            nc.vector.tensor_mul(out=dst, in0=dst, in1=g_pre_t)
                        if use_beta_pre:
                            nc.vector.tensor_add(out=dst, in0=dst, in1=beta_pre_t)
                        nc.vector.tensor_copy(out=xbf, in_=dst)
                    else:
                        nc.vector.tensor_scalar(
                            out=xbf,
                            in0=xts[j],
                            scalar1=mvs[j][:, 0:1],
                            scalar2=r4[:, j : j + 1],
                            op0=ALU.subtract,
                            op1=ALU.mult,
                        )
                    transpose_into(xnT, xbf, i * 128)
                    produce_v(i)
                produce_kT(0, kTs[0], rc)
                if rc < NQB:
                    produce_qT(0, qTs[0], rc)
                if rc >= 1:
                    kcs = range(4 * (rc - 1), 4 * rc)
                    scores_chunk(st00, 0, 0, kcs, kTs[0], qTs[0])
            scores_chunk(st00, 0, 0, range(NT - 4, NT), kTs[0], qTs[0])
            av_finish(st00, 0, 0)

            # remaining qb0 pairs; next pair's kT/qT is produced mid-stream
            # (copies pinned to VectorE so ScalarE stays on exp)
            for pair in range(1, NPAIR):
                st = states[pair][0]
                for rc in range(2):
                    produce_kT(pair, kTs[pair], rc, eng=nc.vector)
                for rc in range(NQB):
                    produce_qT(pair, qTs[pair], rc, eng=nc.vector)
                for seg in range(4):
                    lo = seg * 8
                    if seg >= 1:
                        rc = 2 * seg
                        produce_kT(pair, kTs[pair], rc, eng=nc.vector)
                        produce_kT(pair, kTs[pair], rc + 1, eng=nc.vector)
                    scores_chunk(
                        st, pair, 0, range(lo, lo + 8), kTs[pair], qTs[pair],
                        flush_end=False,
                    )
                av_finish(st, pair, 0)

            phase5([0, 1, 2, 3])

            for pair in range(NPAIR):
                st = states[pair][1]
                scores_chunk(
                    st, pair, 1, range(KC), kTs[pair], qTs[pair], flush_end=False
                )
                av_finish(st, pair, 1)

            phase5([4, 5, 6, 7])

    nc.compile()
    return nc


def kernel(x, w_qkv, b_qkv, g_pre, beta_pre, g_post, beta_post):
    import ml_dtypes
    from concourse.bass_utils import run_bass_kernel_spmd

    x = np.asarray(x, dtype=np.float32)
    w_qkv = np.asarray(w_qkv, dtype=np.float32)
    b_qkv = np.asarray(b_qkv, dtype=np.float32)
    g_pre = np.asarray(g_pre, dtype=np.float32)
    beta_pre = np.asarray(beta_pre, dtype=np.float32)
    g_post = np.asarray(g_post, dtype=np.float32)
    beta_post = np.asarray(beta_post, dtype=np.float32)

    flags = (
        not np.all(g_pre == 1.0),
        not np.all(beta_pre == 0.0),
        not np.all(g_post == 1.0),
        not np.all(beta_post == 0.0),
        not np.all(b_qkv[0:C] == 0.0),
        not np.all(b_qkv[2 * C : 3 * C] == 0.0),
    )
    # NOTE: b_qkv[C:2C] (the K bias) provably cancels in softmax and is
    # intentionally never applied.
    if flags not in _CACHE:
        _CACHE[flags] = _build(flags)
    nc = _CACHE[flags]

    w_bf = w_qkv.astype(ml_dtypes.bfloat16)
    in_maps = []
    for c in range(8):
        b = c // 4
        r = c % 4
        xrot = np.ascontiguousarray(
            np.concatenate([x[b, r * QR :], x[b, : r * QR]], axis=0)
        )
        in_maps.append(
            {
                "xb": xrot,
                "w_qkv": w_bf,
                "b_qkv": b_qkv,
                "g_pre": g_pre,
                "beta_pre": beta_pre,
                "g_post": g_post,
                "beta_post": beta_post,
            }
        )

    global _last_in_maps
    _last_in_maps = in_maps
    res = run_bass_kernel_spmd(nc, in_maps, core_ids=list(range(8)))
    out = np.empty((B, N, C), dtype=np.float32)
    for c in range(8):
        b = c // 4
        r = c % 4
        out[b, r * QR : (r + 1) * QR] = res.results[c]["out"]
    return out



# revision 3
# speedup vs baseline: 1.0192x; 1.0192x over previous
"""Fused pre-LN + QKV + attention + post-LN + residual kernel for TRN2.

Problem (nn_Attention_86517821210894):
    B=2, N=4096, C=512, H=8, D=64
    xn  = LN(x) ; qkv = xn @ w_qkv + b ; per-(b,h) softmax attention
    val = LN(attn_out) ; out = xn + val

Sharding (8 cores, zero collectives):
    core c -> batch b = c // 4, query-row block r = c % 4 (1024 rows).
    Each core receives x[b] ROTATED so its query block is rows 0:1024
    (softmax and the value sum are permutation-invariant over keys, so
    rotating the key order changes nothing), builds K/V for all 4096
    keys, and produces out[b, r*1024:(r+1)*1024].  The K/V qkv matmul is
    recomputed by the 4 cores of a batch; this trades ~30% extra PE work
    for zero inter-core communication.

Device pipeline per core (PE-throughput oriented: the Tensor engine is
the pacing resource at ~300us of matmul streaming; ScalarE's exp stream
is ~260us; the emission order keeps both fed continuously and avoids
HAM clock-down windows):
    1. warmup matmul burst on a zeroed dummy (no data deps) so the PE
       HAM reaches K=8/8 before real work, and stays warm through the
       LN prefix
    2. pre-LN: bn_stats/bn_aggr on VectorE, batched rsqrt via the exp
       table, normalize straight to bf16 (the bf16 rows double as the
       residual source), PE-transpose into xnT [C, N] bf16
    3. qkv from xnT: v [N, 8, 65] (ones column -> AV also computes the
       softmax denominator), kT per head pair, qT for the query block
    4. scoresT[k, q] = kT-slice.T @ qT-slice, two heads packed in the
       PE array (row groups 0-63/64-127 run concurrently), grouped 3
       slots per PSUM tile so each ScalarE exp covers 1536 elem/part.  Max-subtraction is skipped:
       scores*0.125 has |z| < ~2 for LN'd inputs so exp is in range;
       softmax is shift-invariant so the result matches the reference.
    5. AV: valT_aug[65, q] accumulated over key chunks in PSUM
    6. epilogue per (pair, qb) is split and pipelined into the NEXT
       stream: pav->SBUF copies (frees PSUM) at the stream boundary,
       bf16 PE transposes + per-head divide a few chunks later, and the
       post-LN + residual + DMA chain one segment after that.
"""

import sys

sys.path.insert(0, "/opt/trn_rl_repo")

import numpy as np

B, N, C, H = 2, 4096, 512, 8
D = C // H
QR = N // 4  # query rows per core
EPS = 1e-5
SCALE = float(D) ** -0.5

_CACHE = {}


def _build(flags):
    (use_g_pre, use_beta_pre, use_g_post, use_beta_post, use_b_q, use_b_v) = flags

    import concourse.bacc as bacc
    import concourse.bass as bass
    import concourse.tile as tile
    from concourse import mybir
    from concourse.masks import make_identity

    f32 = mybir.dt.float32
    bf16 = mybir.dt.bfloat16
    AF = mybir.ActivationFunctionType
    ALU = mybir.AluOpType

    nc = bacc.Bacc(
        "TRN2", target_bir_lowering=False, debug=False, enable_asserts=False
    )

    xb = nc.dram_tensor("xb", [N, C], f32, kind="ExternalInput").ap()
    w = nc.dram_tensor("w_qkv", [C, 3 * C], bf16, kind="ExternalInput").ap()
    bqkv = nc.dram_tensor("b_qkv", [3 * C], f32, kind="ExternalInput").ap()
    g_pre = nc.dram_tensor("g_pre", [C], f32, kind="ExternalInput").ap()
    beta_pre = nc.dram_tensor("beta_pre", [C], f32, kind="ExternalInput").ap()
    g_post = nc.dram_tensor("g_post", [C], f32, kind="ExternalInput").ap()
    beta_post = nc.dram_tensor("beta_post", [C], f32, kind="ExternalInput").ap()
    out = nc.dram_tensor("out", [QR, C], f32, kind="ExternalOutput").ap()

    NT = N // 128  # 32 row tiles of x[b]
    QT = QR // 128  # 8 row tiles of the query block
    CCH = C // 128  # 4 contraction chunks
    KC = N // 128  # 32 key chunks
    NPAIR = H // 2
    NQB = QR // 512  # 2 query blocks of 512
    GS = 3  # score slots per exp group

    def bcast(vec_ap, p):
        return bass.AP(
            tensor=vec_ap.tensor, offset=vec_ap.offset, ap=[[0, p], *vec_ap.ap]
        )

    with tile.TileContext(nc) as tc:
        with (
            tc.tile_pool(name="consts", bufs=1) as consts,
            tc.tile_pool(name="ln_in", bufs=4) as ln_in,
            tc.tile_pool(name="stats", bufs=8) as stats,
            tc.tile_pool(name="resid", bufs=1) as res_pool,
            tc.tile_pool(name="xnT", bufs=1) as xnT_pool,
            tc.tile_pool(name="vsb", bufs=1) as v_pool,
            tc.tile_pool(name="kT", bufs=4) as kT_pool,
            tc.tile_pool(name="qT", bufs=4) as qT_pool,
            tc.tile_pool(name="expT", bufs=3) as expT_pool,
            tc.tile_pool(name="valT", bufs=4) as valT_pool,
            tc.tile_pool(name="valasm", bufs=1) as val_pool,
            tc.tile_pool(name="outp", bufs=1) as out_pool,
            tc.tile_pool(name="ps3", bufs=2, space="PSUM") as ps3,
            tc.tile_pool(name="pav", bufs=2, space="PSUM") as psum_av,
        ):
            # ---- warmup burst: bring the PE HAM to K=8/8 immediately and
            # keep it warm through the LN prefix (no data dependencies) ----
            dummy = consts.tile([128, 512], bf16)
            nc.vector.memset(dummy, 0.0)
            pw = ps3.tile([128, 3, 512], f32, tag="ps3")
            for _ in range(20):
                nc.tensor.matmul(pw[:, 0, :], dummy[:, 0:128], dummy)
            del pw

            seed_b = consts.tile([128, 1], f32)
            nc.vector.memset(seed_b, 0.5 * 0.6931471805599453 * 127.0)
            # trigger the exp ACT_TABLE_LOAD right away (one-time ~2.7us)
            tbl_warm = stats.tile([128, 1], f32, tag="tblw")
            nc.scalar.activation(out=tbl_warm, in_=seed_b, func=AF.Exp, scale=0.01)

            # ---- x row tiles: first DMAs in the queue so LN starts early ----
            xt_tiles = {}

            def fetch(i):
                xt = ln_in.tile([128, C], f32, tag="xt", name=f"xt{i}")
                nc.sync.dma_start(out=xt, in_=xb[i * 128 : (i + 1) * 128, :])
                xt_tiles[i] = xt

            for i in range(4):
                fetch(i)

            # ---- constants ----
            ident = consts.tile([128, 128], f32)
            make_identity(nc, ident)
            ident_bf = consts.tile([128, 128], bf16)
            make_identity(nc, ident_bf)

            w_sb = consts.tile([128, CCH, 3 * C], bf16)
            nc.sync.dma_start(out=w_sb, in_=w.rearrange("(cc p) m -> p cc m", p=128))

            g_pre_t = beta_pre_t = g_post_t = beta_post_t = None
            if use_g_pre:
                g_pre_t = consts.tile([128, C], f32)
                nc.sync.dma_start(out=g_pre_t, in_=bcast(g_pre, 128))
            if use_beta_pre:
                beta_pre_t = consts.tile([128, C], f32)
                nc.sync.dma_start(out=beta_pre_t, in_=bcast(beta_pre, 128))
            if use_g_post:
                g_post_t = consts.tile([128, C], f32)
                nc.sync.dma_start(out=g_post_t, in_=bcast(g_post, 128))
            if use_beta_post:
                beta_post_t = consts.tile([128, C], f32)
                nc.sync.dma_start(out=beta_post_t, in_=bcast(beta_post, 128))
            bq_t = None
            if use_b_q:
                bq_t = consts.tile([128, CCH, 1], f32)
                nc.sync.dma_start(
                    out=bq_t, in_=bqkv[0:C].rearrange("(cc p) -> p cc 1", p=128)
                )
            bv_t = None
            if use_b_v:
                bv_t = consts.tile([128, C], f32)
                nc.sync.dma_start(out=bv_t, in_=bcast(bqkv[2 * C : 3 * C], 128))

            # ---- persistent tensors ----
            res = res_pool.tile([128, QT, C], bf16)  # bf16 xn rows: residual src
            xnT = xnT_pool.tile([128, CCH, N], bf16)
            v_sb = v_pool.tile([128, KC, H, D + 1], bf16)
            val_asm = val_pool.tile([128, QT, H, D + 1], f32)

            nc.vector.memset(v_sb[:, :, :, D : D + 1], 1.0)

            def rsqrt_into(dst, a4, w, tag):
                """dst = 1/sqrt(a4), a4 > 0, [128, w] f32.

                Bit-trick log2 seed evaluated through ScalarE Exp (the
                resident ACT table) + 2 Newton iterations on VectorE."""
                ai = a4.bitcast(mybir.dt.int32)
                fi = stats.tile([128, w], f32, tag=tag + "_f")
                nc.vector.tensor_copy(out=fi, in_=ai)
                nc.scalar.activation(
                    out=dst,
                    in_=fi,
                    func=AF.Exp,
                    scale=-0.5 * 0.6931471805599453 / 8388608.0,
                    bias=seed_b,
                )
                for _ in range(2):
                    t = stats.tile([128, w], f32, tag=tag + "_t")
                    nc.vector.tensor_mul(out=t, in0=dst, in1=dst)
                    nc.vector.tensor_mul(out=t, in0=t, in1=a4)
                    nc.vector.tensor_scalar(
                        out=t,
                        in0=t,
                        scalar1=-0.5,
                        scalar2=1.5,
                        op0=ALU.mult,
                        op1=ALU.add,
                    )
                    nc.vector.tensor_mul(out=dst, in0=dst, in1=t)

            def transpose_into(dstT, src, col0):
                # src is bf16: single-pass PE transpose
                ps = ps3.tile([128, 512], bf16, tag="ps3")
                pview = ps.rearrange("p (c n) -> p c n", n=128)
                for cc in range(CCH):
                    nc.tensor.transpose(
                        pview[:, cc, :], src[:, cc * 128 : (cc + 1) * 128], ident_bf
                    )
                nc.vector.tensor_copy(out=dstT[:, :, col0 : col0 + 128], in_=pview)

            def produce_v(kc):
                pv = ps3.tile([128, 512], f32, tag="ps3")
                for cc in range(CCH):
                    nc.tensor.matmul(
                        pv,
                        xnT[:, cc, kc * 128 : (kc + 1) * 128],
                        w_sb[:, cc, 2 * C : 3 * C],
                        start=(cc == 0),
                        stop=(cc == CCH - 1),
                    )
                src = pv.rearrange("p (h d) -> p h d", d=D)
                dst = v_sb[:, kc, :, 0:D]
                if use_b_v:
                    nc.vector.tensor_add(
                        out=dst, in0=src, in1=bv_t.rearrange("p (h d) -> p h d", d=D)
                    )
                else:
                    nc.scalar.copy(out=dst, in_=src)

            def produce_kT(pair, kT, rc, eng=None):
                pk = ps3.tile([128, 512], f32, tag="ps3")
                for cc in range(CCH):
                    nc.tensor.matmul(
                        pk,
                        w_sb[:, cc, C + pair * 128 : C + (pair + 1) * 128],
                        xnT[:, cc, rc * 512 : (rc + 1) * 512],
                        start=(cc == 0),
                        stop=(cc == CCH - 1),
                    )
                (eng or nc.vector).tensor_copy(
                    out=kT[:, rc * 512 : (rc + 1) * 512], in_=pk
                )

            def produce_qT(pair, qT, rc, eng=None):
                pq = ps3.tile([128, 512], f32, tag="ps3")
                for cc in range(CCH):
                    nc.tensor.matmul(
                        pq,
                        w_sb[:, cc, pair * 128 : (pair + 1) * 128],
                        xnT[:, cc, rc * 512 : (rc + 1) * 512],
                        start=(cc == 0),
                        stop=(cc == CCH - 1),
                    )
                if use_b_q:
                    (eng or nc.vector).tensor_scalar_add(
                        out=qT[:, rc * 512 : (rc + 1) * 512],
                        in0=pq,
                        scalar1=bq_t[:, pair, :],
                    )
                else:
                    (eng or nc.vector).tensor_copy(
                        out=qT[:, rc * 512 : (rc + 1) * 512], in_=pq
                    )

            # ---- attention slice machinery ----
            class AttState:
                def __init__(self):
                    self.group = None
                    self.pos = 0
                    self.pending = []
                    self.exp_of = {}
                    self.pavs = None
                    self.av_next = 0
                    self.vts = None

            def flush(st):
                ex = expT_pool.tile([128, GS, 512], bf16, tag="expT")
                nc.scalar.activation(
                    out=ex[:, 0 : st.pos, :],
                    in_=st.group[:, 0 : st.pos, :],
                    func=AF.Exp,
                    scale=SCALE,
                )
                for key, p in st.pending:
                    st.exp_of[key] = (ex, p)
                st.pending.clear()
                st.group = None
                st.pos = 0

            def av_drain(st, pair):
                if st.pavs is None:
                    pav_lo = psum_av.tile([128, 512], f32, tag="pav")
                    pav_hi = psum_av.tile([128, 512], f32, tag="pav")
                    st.pavs = (pav_lo, pav_hi)
                while st.av_next < KC and (st.av_next, 1) in st.exp_of:
                    kc = st.av_next
                    for h_idx in range(2):
                        ex, p = st.exp_of.pop((kc, h_idx))
                        nc.tensor.matmul(
                            st.pavs[h_idx][0 : D + 1, :],
                            v_sb[:, kc, 2 * pair + h_idx, :],
                            ex[:, p, :],
                            start=(kc == 0),
                            stop=(kc == KC - 1),
                        )
                    st.av_next += 1

            def scores_chunk(st, pair, qb, kcs, kT, qT, flush_end=True):
                for kc in kcs:
                    for h_idx in range(2):
                        if st.group is None:
                            st.group = ps3.tile([128, GS, 512], f32, tag="ps3")
                            st.pos = 0
                        base = h_idx * 64
                        nc.tensor.matmul(
                            st.group[:, st.pos, :],
                            kT[base : base + 64, kc * 128 : (kc + 1) * 128],
                            qT[base : base + 64, qb * 512 : (qb + 1) * 512],
                        )
                        st.pending.append(((kc, h_idx), st.pos))
                        st.pos += 1
                        if st.pos == GS:
                            flush(st)
                            av_drain(st, pair)
                if flush_end and st.group is not None:
                    flush(st)
                    av_drain(st, pair)

            def av_copy_out(st, pair):
                """End of a (pair, qb) stream: last flush + AV matmuls, then
                move the PSUM accumulators to SBUF so the pav banks free up
                for the next stream."""
                if st.group is not None:
                    flush(st)
                av_drain(st, pair)
                assert st.av_next == KC
                vts = []
                for h_idx in range(2):
                    vt = valT_pool.tile([D + 1, 512], bf16, tag="valT")
                    nc.vector.tensor_copy(out=vt, in_=st.pavs[h_idx][0 : D + 1, :])
                    vts.append(vt)
                st.pavs = None
                st.vts = vts

            def av_transpose(st, pair, qb):
                """Deferred epilogue: bf16 PE transposes of the two valT
                tiles into val_asm row-major (emitted a few chunks into the
                NEXT stream so the PE never runs dry at the boundary)."""
                for h_idx in range(2):
                    h = 2 * pair + h_idx
                    vt = st.vts[h_idx]
                    for j in range(4):
                        pt = ps3.tile([128, D + 1], bf16, tag="ps3")
                        nc.tensor.transpose(
                            pt,
                            vt[:, j * 128 : (j + 1) * 128],
                            ident_bf[0 : D + 1, 0 : D + 1],
                        )
                        nc.vector.tensor_copy(
                            out=val_asm[:, qb * 4 + j, h, :], in_=pt
                        )
                st.vts = None

            # ---- phase 5, split ----
            ot_tiles = {}

            def phase5_div(pair, qb):
                for qtile in range(qb * 4, qb * 4 + 4):
                    if qtile not in ot_tiles:
                        ot_tiles[qtile] = out_pool.tile(
                            [128, C], f32, tag=f"ot{qtile}", name=f"ot{qtile}"
                        )
                    ot = ot_tiles[qtile]
                    va = val_asm[:, qtile]
                    for h_idx in range(2):
                        h = 2 * pair + h_idx
                        rs = stats.tile([128, 1], f32, tag="rs")
                        nc.vector.reciprocal(out=rs, in_=va[:, h, D : D + 1])
                        nc.vector.tensor_scalar_mul(
                            out=ot[:, h * D : (h + 1) * D],
                            in0=va[:, h, 0:D],
                            scalar1=rs,
                        )

            def phase5_ln(qtiles):
                m4 = stats.tile([128, 4], f32, tag="m4b")
                a4b = stats.tile([128, 4], f32, tag="a4b")
                r4 = stats.tile([128, 4], f32, tag="r4b")
                for jj, qtile in enumerate(qtiles):
                    ot = ot_tiles[qtile]
                    if use_b_v:
                        nc.vector.tensor_add(out=ot, in0=ot, in1=bv_t)
                    st6 = stats.tile([128, 6], f32, tag="bn6")
                    nc.vector.bn_stats(out=st6, in_=ot)
                    mv = stats.tile([128, 2], f32, tag="mv")
                    nc.vector.bn_aggr(out=mv, in_=st6)
                    nc.vector.tensor_copy(out=m4[:, jj : jj + 1], in_=mv[:, 0:1])
                    nc.vector.tensor_copy(out=a4b[:, jj : jj + 1], in_=mv[:, 1:2])
                nc.vector.tensor_scalar_add(out=a4b, in0=a4b, scalar1=EPS)
                rsqrt_into(r4, a4b, 4, "p5")
                for jj, qtile in enumerate(qtiles):
                    ot = ot_tiles[qtile]
                    nc.vector.tensor_scalar(
                        out=ot,
                        in0=ot,
                        scalar1=m4[:, jj : jj + 1],
                        scalar2=r4[:, jj : jj + 1],
                        op0=ALU.subtract,
                        op1=ALU.mult,
                    )
                    if use_g_post:
                        nc.vector.tensor_mul(out=ot, in0=ot, in1=g_post_t)
                    if use_beta_post:
                        nc.vector.tensor_add(out=ot, in0=ot, in1=beta_post_t)
                    nc.vector.tensor_add(out=ot, in0=ot, in1=res[:, qtile, :])
                    nc.sync.dma_start(
                        out=out[qtile * 128 : (qtile + 1) * 128, :], in_=ot
                    )

            # ================= emission =================
            kTs = [
                kT_pool.tile([128, N], bf16, tag="kT", name=f"kT{p}")
                for p in range(NPAIR)
            ]
            qTs = [
                qT_pool.tile([128, QR], bf16, tag="qT", name=f"qT{p}")
                for p in range(NPAIR)
            ]
            states = [[AttState() for _ in range(NQB)] for _ in range(NPAIR)]

            # phase 1+2 fused; first two LN groups are 2 tiles for latency,
            # pair-0 qb-0 scores+exp+AV trickle one column-group behind
            st00 = states[0][0]
            groups = [[0, 1], [2, 3]] + [
                list(range(4 * g, 4 * g + 4)) for g in range(1, NT // 4)
            ]
            rc_done = 0
            for grp in groups:
                gw = len(grp)
                xts, mvs = [], []
                for i in grp:
                    xt = xt_tiles.pop(i)
                    if i + 4 < NT:
                        fetch(i + 4)
                    st6 = stats.tile([128, 6], f32, tag="bn6")
                    nc.vector.bn_stats(out=st6, in_=xt)
                    mv = stats.tile([128, 2], f32, tag="mv")
                    nc.vector.bn_aggr(out=mv, in_=st6)
                    xts.append(xt)
                    mvs.append(mv)
                a4 = stats.tile([128, gw], f32, tag=f"a4_{gw}")
                for j in range(gw):
                    nc.vector.tensor_copy(out=a4[:, j : j + 1], in_=mvs[j][:, 1:2])
                nc.vector.tensor_scalar_add(out=a4, in0=a4, scalar1=EPS)
                r4 = stats.tile([128, gw], f32, tag=f"r4_{gw}")
                rsqrt_into(r4, a4, gw, f"p1_{gw}")
                for j, i in enumerate(grp):
                    if i < QT:
                        dst = res[:, i, :]
                    else:
                        dst = ln_in.tile([128, C], bf16, tag="xbf")
                    nc.vector.tensor_scalar(
                        out=dst,
                        in0=xts[j],
                        scalar1=mvs[j][:, 0:1],
                        scalar2=r4[:, j : j + 1],
                        op0=ALU.subtract,
                        op1=ALU.mult,
                    )
                    if use_g_pre:
                        nc.vector.tensor_mul(out=dst, in0=dst, in1=g_pre_t)
                    if use_beta_pre:
                        nc.vector.tensor_add(out=dst, in0=dst, in1=beta_pre_t)
                    transpose_into(xnT, dst, i * 128)
                    produce_v(i)
                completed = grp[-1] + 1
                while completed >= 4 * (rc_done + 1):
                    rc = rc_done
                    produce_kT(0, kTs[0], rc)
                    if rc < NQB:
                        produce_qT(0, qTs[0], rc)
                    if rc >= 1:
                        scores_chunk(
                            st00, 0, 0, range(4 * (rc - 1), 4 * rc), kTs[0], qTs[0]
                        )
                    rc_done += 1
            scores_chunk(st00, 0, 0, range(NT - 4, NT), kTs[0], qTs[0])
            av_copy_out(st00, 0)
            prev = (0, 0)

            # remaining qb0 pairs; next pair's kT/qT is produced mid-stream
            # (copies pinned to VectorE so ScalarE stays on exp); the prior
            # stream's epilogue (transposes + divides) lands inside seg 0
            for pair in range(1, NPAIR):
                st = states[pair][0]
                for rc in range(2):
                    produce_kT(pair, kTs[pair], rc, eng=nc.vector)
                for rc in range(NQB):
                    produce_qT(pair, qTs[pair], rc, eng=nc.vector)
                for seg in range(4):
                    lo = seg * 8
                    if seg >= 1:
                        rc = 2 * seg
                        produce_kT(pair, kTs[pair], rc, eng=nc.vector)
                        produce_kT(pair, kTs[pair], rc + 1, eng=nc.vector)
                    scores_chunk(
                        st, pair, 0, range(lo, lo + 8), kTs[pair], qTs[pair],
                        flush_end=False,
                    )
                    if seg == 0:
                        pp, pq = prev
                        av_transpose(states[pp][pq], pp, pq)
                        phase5_div(pp, pq)
                av_copy_out(st, pair)
                prev = (pair, 0)

            # qb1 streams; qb0's last epilogue + post-LN chains overlap the
            # early segments so the PE and ScalarE never drain
            for pair in range(NPAIR):
                st = states[pair][1]
                for seg in range(4):
                    lo = seg * 8
                    scores_chunk(
                        st, pair, 1, range(lo, lo + 8), kTs[pair], qTs[pair],
                        flush_end=False,
                    )
                    if seg == 0:
                        pp, pq = prev
                        av_transpose(states[pp][pq], pp, pq)
                        phase5_div(pp, pq)
                    if seg == 1 and pair == 0:
                        phase5_ln([0, 1, 2, 3])
                av_copy_out(st, pair)
                prev = (pair, 1)

            av_transpose(states[3][1], 3, 1)
            phase5_div(3, 1)
            phase5_ln([4, 5, 6, 7])

    nc.compile()
    return nc


def kernel(x, w_qkv, b_qkv, g_pre, beta_pre, g_post, beta_post):
    import ml_dtypes
    from concourse.bass_utils import run_bass_kernel_spmd

    x = np.asarray(x, dtype=np.float32)
    w_qkv = np.asarray(w_qkv, dtype=np.float32)
    b_qkv = np.asarray(b_qkv, dtype=np.float32)
    g_pre = np.asarray(g_pre, dtype=np.float32)
    beta_pre = np.asarray(beta_pre, dtype=np.float32)
    g_post = np.asarray(g_post, dtype=np.float32)
    beta_post = np.asarray(beta_post, dtype=np.float32)

    flags = (
        not np.all(g_pre == 1.0),
        not np.all(beta_pre == 0.0),
        not np.all(g_post == 1.0),
        not np.all(beta_post == 0.0),
        not np.all(b_qkv[0:C] == 0.0),
        not np.all(b_qkv[2 * C : 3 * C] == 0.0),
    )
    # NOTE: b_qkv[C:2C] (the K bias) provably cancels in softmax and is
    # intentionally never applied.
    if flags not in _CACHE:
        _CACHE[flags] = _build(flags)
    nc = _CACHE[flags]

    w_bf = w_qkv.astype(ml_dtypes.bfloat16)
    in_maps = []
    for c in range(8):
        b = c // 4
        r = c % 4
        xrot = np.ascontiguousarray(
            np.concatenate([x[b, r * QR :], x[b, : r * QR]], axis=0)
        )
        in_maps.append(
            {
                "xb": xrot,
                "w_qkv": w_bf,
                "b_qkv": b_qkv,
                "g_pre": g_pre,
                "beta_pre": beta_pre,
                "g_post": g_post,
                "beta_post": beta_post,
            }
        )

    global _last_in_maps
    _last_in_maps = in_maps
    res = run_bass_kernel_spmd(nc, in_maps, core_ids=list(range(8)))
    out = np.empty((B, N, C), dtype=np.float32)
    for c in range(8):
        b = c // 4
        r = c % 4
        out[b, r * QR : (r + 1) * QR] = res.results[c]["out"]
    return out


# revision 6
# speedup vs baseline: 1.1118x; 1.0908x over previous
"""Fused pre-LN + QKV + attention + post-LN + residual kernel for TRN2.

Problem (nn_Attention_86517821210894):
    B=2, N=4096, C=512, H=8, D=64
    xn  = LN(x) ; qkv = xn @ w_qkv + b ; per-(b,h) softmax attention
    val = LN(attn_out) ; out = xn + val

Sharding (8 cores, zero collectives):
    core c -> batch b = c // 4, query-row block r = c % 4 (1024 rows).
    Each core receives x[b] ROTATED so its query block is rows 0:1024
    (softmax and the value sum are permutation-invariant over keys, so
    rotating the key order changes nothing), builds K/V for all 4096
    keys, and produces out[b, r*1024:(r+1)*1024].  The K/V qkv matmul is
    recomputed by the 4 cores of a batch; this trades ~30% extra PE work
    for zero inter-core communication.

Device pipeline per core (PE-throughput oriented: the Tensor engine is
the pacing resource at ~300us of matmul streaming; ScalarE's exp stream
is ~260us; the emission order keeps both fed continuously and avoids
HAM clock-down windows):
    1. warmup matmul burst on a zeroed dummy (no data deps) so the PE
       HAM reaches K=8/8 before real work, and stays warm through the
       LN prefix
    2. pre-LN: bn_stats/bn_aggr on VectorE, batched rsqrt via the exp
       table, normalize straight to bf16 (the bf16 rows double as the
       residual source), PE-transpose into xnT [C, N] bf16
    3. qkv from xnT: v [N, 8, 65] (ones column -> AV also computes the
       softmax denominator), kT per head pair, qT for the query block
    4. scoresT[k, q] = kT-slice.T @ qT-slice, two heads packed in the
       PE array (row groups 0-63/64-127 run concurrently), grouped 3
       slots per PSUM tile so each ScalarE exp covers 1536 elem/part.  Max-subtraction is skipped:
       scores*0.125 has |z| < ~2 for LN'd inputs so exp is in range;
       softmax is shift-invariant so the result matches the reference.
    5. AV: valT_aug[65, q] accumulated over key chunks in PSUM
    6. epilogue per (pair, qb) is split and pipelined into the NEXT
       stream: pav->SBUF copies (frees PSUM) at the stream boundary,
       bf16 PE transposes + per-head divide a few chunks later, and the
       post-LN + residual + DMA chain one segment after that.
"""

import sys

sys.path.insert(0, "/opt/trn_rl_repo")

import numpy as np

B, N, C, H = 2, 4096, 512, 8
D = C // H
QR = N // 4  # query rows per core
EPS = 1e-5
SCALE = float(D) ** -0.5

_CACHE = {}


def _build(flags):
    (use_g_pre, use_beta_pre, use_g_post, use_beta_post, use_b_q, use_b_v) = flags

    import concourse.bacc as bacc
    import concourse.bass as bass
    import concourse.tile as tile
    from concourse import mybir
    from concourse.masks import make_identity

    f32 = mybir.dt.float32
    bf16 = mybir.dt.bfloat16
    AF = mybir.ActivationFunctionType
    ALU = mybir.AluOpType

    nc = bacc.Bacc(
        "TRN2", target_bir_lowering=False, debug=False, enable_asserts=False
    )

    xb = nc.dram_tensor("xb", [N, C], f32, kind="ExternalInput").ap()
    w = nc.dram_tensor("w_qkv", [C, 3 * C], bf16, kind="ExternalInput").ap()
    bqkv = nc.dram_tensor("b_qkv", [3 * C], f32, kind="ExternalInput").ap()
    g_pre = nc.dram_tensor("g_pre", [C], f32, kind="ExternalInput").ap()
    beta_pre = nc.dram_tensor("beta_pre", [C], f32, kind="ExternalInput").ap()
    g_post = nc.dram_tensor("g_post", [C], f32, kind="ExternalInput").ap()
    beta_post = nc.dram_tensor("beta_post", [C], f32, kind="ExternalInput").ap()
    out = nc.dram_tensor("out", [QR, C], f32, kind="ExternalOutput").ap()

    NT = N // 128  # 32 row tiles of x[b]
    QT = QR // 128  # 8 row tiles of the query block
    CCH = C // 128  # 4 contraction chunks
    KC = N // 128  # 32 key chunks
    NPAIR = H // 2
    NQB = QR // 512  # 2 query blocks of 512
    GS = 3  # score slots per exp group

    def bcast(vec_ap, p):
        return bass.AP(
            tensor=vec_ap.tensor, offset=vec_ap.offset, ap=[[0, p], *vec_ap.ap]
        )

    with tile.TileContext(nc) as tc:
        with (
            tc.tile_pool(name="consts", bufs=1) as consts,
            tc.tile_pool(name="ln_in", bufs=4) as ln_in,
            tc.tile_pool(name="stats", bufs=8) as stats,
            tc.tile_pool(name="resid", bufs=1) as res_pool,
            tc.tile_pool(name="xnT", bufs=1) as xnT_pool,
            tc.tile_pool(name="vsb", bufs=1) as v_pool,
            tc.tile_pool(name="kT", bufs=4) as kT_pool,
            tc.tile_pool(name="qT", bufs=4) as qT_pool,
            tc.tile_pool(name="expT", bufs=3) as expT_pool,
            tc.tile_pool(name="valT", bufs=4) as valT_pool,
            tc.tile_pool(name="valasm", bufs=1) as val_pool,
            tc.tile_pool(name="outp", bufs=1) as out_pool,
            tc.tile_pool(name="ps3", bufs=2, space="PSUM") as ps3,
            tc.tile_pool(name="pav", bufs=2, space="PSUM") as psum_av,
        ):
            # ---- warmup burst: bring the PE HAM to K=8/8 immediately and
            # keep it warm through the LN prefix (no data dependencies) ----
            dummy = consts.tile([128, 512], bf16)
            nc.vector.memset(dummy, 0.0)
            pw = ps3.tile([128, 3, 512], f32, tag="ps3")
            for _ in range(26):
                nc.tensor.matmul(pw[:, 0, :], dummy[:, 0:128], dummy)
            del pw

            seed_b = consts.tile([128, 1], f32)
            nc.vector.memset(seed_b, 0.5 * 0.6931471805599453 * 127.0)
            # trigger the exp ACT_TABLE_LOAD right away (one-time ~2.7us)
            tbl_warm = stats.tile([128, 1], f32, tag="tblw")
            nc.scalar.activation(out=tbl_warm, in_=seed_b, func=AF.Exp, scale=0.01)

            # ---- x row tiles: first DMAs in the queue so LN starts early ----
            xt_tiles = {}

            def fetch(i):
                xt = ln_in.tile([128, C], f32, tag="xt", name=f"xt{i}")
                nc.sync.dma_start(out=xt, in_=xb[i * 128 : (i + 1) * 128, :])
                xt_tiles[i] = xt

            for i in range(4):
                fetch(i)

            # ---- constants ----
            ident = consts.tile([128, 128], f32)
            make_identity(nc, ident)
            ident_bf = consts.tile([128, 128], bf16)
            make_identity(nc, ident_bf)

            w_sb = consts.tile([128, CCH, 3 * C], bf16)
            nc.sync.dma_start(out=w_sb, in_=w.rearrange("(cc p) m -> p cc m", p=128))

            g_pre_t = beta_pre_t = g_post_t = beta_post_t = None
            if use_g_pre:
                g_pre_t = consts.tile([128, C], f32)
                nc.sync.dma_start(out=g_pre_t, in_=bcast(g_pre, 128))
            if use_beta_pre:
                beta_pre_t = consts.tile([128, C], f32)
                nc.sync.dma_start(out=beta_pre_t, in_=bcast(beta_pre, 128))
            if use_g_post:
                g_post_t = consts.tile([128, C], f32)
                nc.sync.dma_start(out=g_post_t, in_=bcast(g_post, 128))
            if use_beta_post:
                beta_post_t = consts.tile([128, C], f32)
                nc.sync.dma_start(out=beta_post_t, in_=bcast(beta_post, 128))
            bq_t = None
            if use_b_q:
                bq_t = consts.tile([128, CCH, 1], f32)
                nc.sync.dma_start(
                    out=bq_t, in_=bqkv[0:C].rearrange("(cc p) -> p cc 1", p=128)
                )
            bv_t = None
            if use_b_v:
                bv_t = consts.tile([128, C], f32)
                nc.sync.dma_start(out=bv_t, in_=bcast(bqkv[2 * C : 3 * C], 128))

            # ---- persistent tensors ----
            res = res_pool.tile([128, QT, C], bf16)  # bf16 xn rows: residual src
            xnT = xnT_pool.tile([128, CCH, N], bf16)
            v_sb = v_pool.tile([128, KC, H, D + 1], bf16)
            val_asm = val_pool.tile([128, QT, H, D + 1], f32)

            nc.vector.memset(v_sb[:, :, :, D : D + 1], 1.0)

            def rsqrt_into(dst, a4, w, tag):
                """dst = 1/sqrt(a4), a4 > 0, [128, w] f32.

                Bit-trick log2 seed evaluated through ScalarE Exp (the
                resident ACT table) + 2 Newton iterations on VectorE."""
                ai = a4.bitcast(mybir.dt.int32)
                fi = stats.tile([128, w], f32, tag=tag + "_f")
                nc.vector.tensor_copy(out=fi, in_=ai)
                nc.scalar.activation(
                    out=dst,
                    in_=fi,
                    func=AF.Exp,
                    scale=-0.5 * 0.6931471805599453 / 8388608.0,
                    bias=seed_b,
                )
                for _ in range(2):
                    t = stats.tile([128, w], f32, tag=tag + "_t")
                    nc.vector.tensor_mul(out=t, in0=dst, in1=dst)
                    nc.vector.tensor_mul(out=t, in0=t, in1=a4)
                    nc.vector.tensor_scalar(
                        out=t,
                        in0=t,
                        scalar1=-0.5,
                        scalar2=1.5,
                        op0=ALU.mult,
                        op1=ALU.add,
                    )
                    nc.vector.tensor_mul(out=dst, in0=dst, in1=t)

            def transpose_into(dstT, src, col0):
                # src is bf16: single-pass PE transpose
                ps = ps3.tile([128, 512], bf16, tag="ps3")
                pview = ps.rearrange("p (c n) -> p c n", n=128)
                for cc in range(CCH):
                    nc.tensor.transpose(
                        pview[:, cc, :], src[:, cc * 128 : (cc + 1) * 128], ident_bf
                    )
                nc.vector.tensor_copy(out=dstT[:, :, col0 : col0 + 128], in_=pview)

            def produce_v(kc):
                pv = ps3.tile([128, 512], f32, tag="ps3")
                for cc in range(CCH):
                    nc.tensor.matmul(
                        pv,
                        xnT[:, cc, kc * 128 : (kc + 1) * 128],
                        w_sb[:, cc, 2 * C : 3 * C],
                        start=(cc == 0),
                        stop=(cc == CCH - 1),
                    )
                src = pv.rearrange("p (h d) -> p h d", d=D)
                dst = v_sb[:, kc, :, 0:D]
                if use_b_v:
                    nc.vector.tensor_add(
                        out=dst, in0=src, in1=bv_t.rearrange("p (h d) -> p h d", d=D)
                    )
                else:
                    nc.scalar.copy(out=dst, in_=src)

            def produce_kT(pair, kT, rc, eng=None):
                pk = ps3.tile([128, 512], f32, tag="ps3")
                for cc in range(CCH):
                    nc.tensor.matmul(
                        pk,
                        w_sb[:, cc, C + pair * 128 : C + (pair + 1) * 128],
                        xnT[:, cc, rc * 512 : (rc + 1) * 512],
                        start=(cc == 0),
                        stop=(cc == CCH - 1),
                    )
                (eng or nc.vector).tensor_copy(
                    out=kT[:, rc * 512 : (rc + 1) * 512], in_=pk
                )

            def produce_qT(pair, qT, rc, eng=None):
                pq = ps3.tile([128, 512], f32, tag="ps3")
                for cc in range(CCH):
                    nc.tensor.matmul(
                        pq,
                        w_sb[:, cc, pair * 128 : (pair + 1) * 128],
                        xnT[:, cc, rc * 512 : (rc + 1) * 512],
                        start=(cc == 0),
                        stop=(cc == CCH - 1),
                    )
                if use_b_q:
                    (eng or nc.vector).tensor_scalar_add(
                        out=qT[:, rc * 512 : (rc + 1) * 512],
                        in0=pq,
                        scalar1=bq_t[:, pair, :],
                    )
                else:
                    (eng or nc.vector).tensor_copy(
                        out=qT[:, rc * 512 : (rc + 1) * 512], in_=pq
                    )

            # ---- attention slice machinery ----
            class AttState:
                def __init__(self):
                    self.group = None
                    self.pos = 0
                    self.pending = []
                    self.exp_of = {}
                    self.pavs = None
                    self.av_next = 0
                    self.vts = None

            def flush(st):
                ex = expT_pool.tile([128, GS, 512], bf16, tag="expT")
                nc.scalar.activation(
                    out=ex[:, 0 : st.pos, :],
                    in_=st.group[:, 0 : st.pos, :],
                    func=AF.Exp,
                    scale=SCALE,
                )
                for key, p in st.pending:
                    st.exp_of[key] = (ex, p)
                st.pending.clear()
                st.group = None
                st.pos = 0

            def av_drain(st, pair):
                if st.pavs is None:
                    pav_lo = psum_av.tile([128, 512], f32, tag="pav")
                    pav_hi = psum_av.tile([128, 512], f32, tag="pav")
                    st.pavs = (pav_lo, pav_hi)
                while st.av_next < KC and (st.av_next, 1) in st.exp_of:
                    kc = st.av_next
                    for h_idx in range(2):
                        ex, p = st.exp_of.pop((kc, h_idx))
                        nc.tensor.matmul(
                            st.pavs[h_idx][0 : D + 1, :],
                            v_sb[:, kc, 2 * pair + h_idx, :],
                            ex[:, p, :],
                            start=(kc == 0),
                            stop=(kc == KC - 1),
                        )
                    st.av_next += 1

            def scores_chunk(st, pair, qb, kcs, kT, qT, flush_end=True):
                for kc in kcs:
                    for h_idx in range(2):
                        if st.group is None:
                            st.group = ps3.tile([128, GS, 512], f32, tag="ps3")
                            st.pos = 0
                        base = h_idx * 64
                        nc.tensor.matmul(
                            st.group[:, st.pos, :],
                            kT[base : base + 64, kc * 128 : (kc + 1) * 128],
                            qT[base : base + 64, qb * 512 : (qb + 1) * 512],
                        )
                        st.pending.append(((kc, h_idx), st.pos))
                        st.pos += 1
                        if st.pos == GS:
                            # drain AV for exps from EARLIER flushes first so
                            # the PE never waits on the exp it just requested
                            av_drain(st, pair)
                            flush(st)
                if flush_end and st.group is not None:
                    av_drain(st, pair)
                    flush(st)

            def stream_close(st, pair):
                # close the open score group at end-of-stream so later ps3
                # allocations can never wedge behind an unflushed group
                av_drain(st, pair)
                if st.group is not None:
                    flush(st)

            def av_copy_out(st, pair):
                """End of a (pair, qb) stream: last flush + AV matmuls, then
                move the PSUM accumulators to SBUF so the pav banks free up
                for the next stream."""
                av_drain(st, pair)
                if st.group is not None:
                    flush(st)
                av_drain(st, pair)
                assert st.av_next == KC
                vts = []
                for h_idx in range(2):
                    vt = valT_pool.tile([D + 1, 512], bf16, tag="valT")
                    nc.vector.tensor_copy(out=vt, in_=st.pavs[h_idx][0 : D + 1, :])
                    vts.append(vt)
                st.pavs = None
                st.vts = vts

            def av_transpose(st, pair, qb, h_idx):
                """Deferred epilogue: bf16 PE transposes of one valT tile
                into val_asm row-major (emitted a few chunks into the NEXT
                stream so the PE never runs dry at the boundary)."""
                h = 2 * pair + h_idx
                vt = st.vts[h_idx]
                for j in range(4):
                    pt = ps3.tile([128, D + 1], bf16, tag="ps3")
                    nc.tensor.transpose(
                        pt,
                        vt[:, j * 128 : (j + 1) * 128],
                        ident_bf[0 : D + 1, 0 : D + 1],
                    )
                    nc.vector.tensor_copy(
                        out=val_asm[:, qb * 4 + j, h, :], in_=pt
                    )

            # ---- phase 5, split ----
            ot_tiles = {}

            def phase5_div(pair, qb):
                for qtile in range(qb * 4, qb * 4 + 4):
                    if qtile not in ot_tiles:
                        ot_tiles[qtile] = out_pool.tile(
                            [128, C], f32, tag=f"ot{qtile}", name=f"ot{qtile}"
                        )
                    ot = ot_tiles[qtile]
                    va = val_asm[:, qtile]
                    rs = stats.tile([128, 2], f32, tag="rs")
                    nc.vector.reciprocal(
                        out=rs, in_=va[:, 2 * pair : 2 * pair + 2, D : D + 1]
                    )
                    for h_idx in range(2):
                        h = 2 * pair + h_idx
                        nc.vector.tensor_scalar_mul(
                            out=ot[:, h * D : (h + 1) * D],
                            in0=va[:, h, 0:D],
                            scalar1=rs[:, h_idx : h_idx + 1],
                        )

            def phase5_ln(qtiles):
                gw = len(qtiles)
                m4 = stats.tile([128, gw], f32, tag=f"m4b{gw}")
                a4b = stats.tile([128, gw], f32, tag=f"a4b{gw}")
                r4 = stats.tile([128, gw], f32, tag=f"r4b{gw}")
                for jj, qtile in enumerate(qtiles):
                    ot = ot_tiles[qtile]
                    if use_b_v:
                        nc.vector.tensor_add(out=ot, in0=ot, in1=bv_t)
                    st6 = stats.tile([128, 6], f32, tag="bn6")
                    nc.vector.bn_stats(out=st6, in_=ot)
                    mv = stats.tile([128, 2], f32, tag="mv")
                    nc.vector.bn_aggr(out=mv, in_=st6)
                    nc.vector.tensor_copy(out=m4[:, jj : jj + 1], in_=mv[:, 0:1])
                    nc.vector.tensor_copy(out=a4b[:, jj : jj + 1], in_=mv[:, 1:2])
                nc.vector.tensor_scalar_add(out=a4b, in0=a4b, scalar1=EPS)
                rsqrt_into(r4, a4b, gw, f"p5{gw}")
                for jj, qtile in enumerate(qtiles):
                    ot = ot_tiles[qtile]
                    nc.vector.tensor_scalar(
                        out=ot,
                        in0=ot,
                        scalar1=m4[:, jj : jj + 1],
                        scalar2=r4[:, jj : jj + 1],
                        op0=ALU.subtract,
                        op1=ALU.mult,
                    )
                    if use_g_post:
                        nc.vector.tensor_mul(out=ot, in0=ot, in1=g_post_t)
                    if use_beta_post:
                        nc.vector.tensor_add(out=ot, in0=ot, in1=beta_post_t)
                    nc.vector.tensor_add(out=ot, in0=ot, in1=res[:, qtile, :])
                    nc.sync.dma_start(
                        out=out[qtile * 128 : (qtile + 1) * 128, :], in_=ot
                    )

            # ================= emission =================
            kTs = [
                kT_pool.tile([128, N], bf16, tag="kT", name=f"kT{p}")
                for p in range(NPAIR)
            ]
            qTs = [
                qT_pool.tile([128, QR], bf16, tag="qT", name=f"qT{p}")
                for p in range(NPAIR)
            ]
            states = [[AttState() for _ in range(NQB)] for _ in range(NPAIR)]

            # phase 1+2 fused; first two LN groups are 2 tiles for latency,
            # pair-0 qb-0 scores+exp+AV trickle one column-group behind
            st00 = states[0][0]
            groups = [[0, 1], [2, 3]] + [
                list(range(4 * g, 4 * g + 4)) for g in range(1, NT // 4)
            ]
            rc_done = 0
            for grp in groups:
                gw = len(grp)
                xts, mvs = [], []
                for i in grp:
                    xt = xt_tiles.pop(i)
                    if i + 4 < NT:
                        fetch(i + 4)
                    st6 = stats.tile([128, 6], f32, tag="bn6")
                    nc.vector.bn_stats(out=st6, in_=xt)
                    mv = stats.tile([128, 2], f32, tag="mv")
                    nc.vector.bn_aggr(out=mv, in_=st6)
                    xts.append(xt)
                    mvs.append(mv)
                a4 = stats.tile([128, gw], f32, tag=f"a4_{gw}")
                for j in range(gw):
                    nc.vector.tensor_copy(out=a4[:, j : j + 1], in_=mvs[j][:, 1:2])
                nc.vector.tensor_scalar_add(out=a4, in0=a4, scalar1=EPS)
                r4 = stats.tile([128, gw], f32, tag=f"r4_{gw}")
                rsqrt_into(r4, a4, gw, f"p1_{gw}")
                for j, i in enumerate(grp):
                    if i < QT:
                        dst = res[:, i, :]
                    else:
                        dst = ln_in.tile([128, C], bf16, tag="xbf")
                    nc.vector.tensor_scalar(
                        out=dst,
                        in0=xts[j],
                        scalar1=mvs[j][:, 0:1],
                        scalar2=r4[:, j : j + 1],
                        op0=ALU.subtract,
                        op1=ALU.mult,
                    )
                    if use_g_pre:
                        nc.vector.tensor_mul(out=dst, in0=dst, in1=g_pre_t)
                    if use_beta_pre:
                        nc.vector.tensor_add(out=dst, in0=dst, in1=beta_pre_t)
                    transpose_into(xnT, dst, i * 128)
                    produce_v(i)
                completed = grp[-1] + 1
                while completed >= 4 * (rc_done + 1):
                    rc = rc_done
                    produce_kT(0, kTs[0], rc)
                    if rc < NQB:
                        produce_qT(0, qTs[0], rc)
                    if rc >= 1:
                        scores_chunk(
                            st00, 0, 0, range(4 * (rc - 1), 4 * rc), kTs[0], qTs[0]
                        )
                    # produce pair 1's qT / first kT blocks inside the tail of
                    # the phase-1+2 trickle so its stream starts immediately
                    if rc == 5:
                        produce_qT(1, qTs[1], 0, eng=nc.vector)
                    if rc == 6:
                        produce_qT(1, qTs[1], 1, eng=nc.vector)
                    if rc == 7:
                        produce_kT(1, kTs[1], 0, eng=nc.vector)
                    rc_done += 1
            scores_chunk(st00, 0, 0, range(NT - 4, NT), kTs[0], qTs[0])
            produce_kT(1, kTs[1], 1, eng=nc.vector)

            # ---- pipelined (pair, qb) streams ----
            # stream order after (0,0): (1,0) (2,0) (3,0) (0,1) (1,1) (2,1) (3,1)
            # av_copy_out of the previous stream is deferred until after the
            # next stream's first chunk; transposes / divides / post-LN land
            # progressively further in so no engine drains at a boundary.
            order = [(p, 0) for p in range(1, NPAIR)] + [
                (p, 1) for p in range(NPAIR)
            ]
            prev = (0, 0)
            for pair, qb in order:
                st = states[pair][qb]
                pst = states[prev[0]][prev[1]]
                for ci in range(11):  # chunks of 3 kc (exactly 2 exp groups)
                    kcs = range(3 * ci, min(3 * ci + 3, KC))
                    scores_chunk(
                        st, pair, qb, kcs, kTs[pair], qTs[pair], flush_end=False
                    )
                    if ci == 0:
                        av_copy_out(pst, prev[0])
                    elif ci == 1:
                        av_transpose(pst, prev[0], prev[1], 0)
                    elif ci == 2:
                        av_transpose(pst, prev[0], prev[1], 1)
                        pst.vts = None
                        phase5_div(prev[0], prev[1])
                    elif ci == 3 and (pair, qb) == (0, 1):
                        phase5_ln([0, 1, 2, 3])
                    if qb == 0:
                        # own kT blocks two chunks ahead; next pair's qT and
                        # first kT blocks in the back half of the stream
                        if 0 <= ci < 6:
                            produce_kT(pair, kTs[pair], ci + 2, eng=nc.vector)
                        if pair + 1 < NPAIR:
                            nxt = pair + 1
                            if ci == 6:
                                produce_qT(nxt, qTs[nxt], 0, eng=nc.vector)
                            elif ci == 7:
                                produce_qT(nxt, qTs[nxt], 1, eng=nc.vector)
                            elif ci == 8:
                                produce_kT(nxt, kTs[nxt], 0, eng=nc.vector)
                            elif ci == 9:
                                produce_kT(nxt, kTs[nxt], 1, eng=nc.vector)
                stream_close(st, pair)
                prev = (pair, qb)

            st31 = states[3][1]
            av_copy_out(st31, 3)
            av_transpose(st31, 3, 1, 0)
            av_transpose(st31, 3, 1, 1)
            st31.vts = None
            phase5_div(3, 1)
            phase5_ln([4, 5])
            phase5_ln([6, 7])

    nc.compile()
    return nc


def kernel(x, w_qkv, b_qkv, g_pre, beta_pre, g_post, beta_post):
    import ml_dtypes
    from concourse.bass_utils import run_bass_kernel_spmd

    x = np.asarray(x, dtype=np.float32)
    w_qkv = np.asarray(w_qkv, dtype=np.float32)
    b_qkv = np.asarray(b_qkv, dtype=np.float32)
    g_pre = np.asarray(g_pre, dtype=np.float32)
    beta_pre = np.asarray(beta_pre, dtype=np.float32)
    g_post = np.asarray(g_post, dtype=np.float32)
    beta_post = np.asarray(beta_post, dtype=np.float32)

    flags = (
        not np.all(g_pre == 1.0),
        not np.all(beta_pre == 0.0),
        not np.all(g_post == 1.0),
        not np.all(beta_post == 0.0),
        not np.all(b_qkv[0:C] == 0.0),
        not np.all(b_qkv[2 * C : 3 * C] == 0.0),
    )
    # NOTE: b_qkv[C:2C] (the K bias) provably cancels in softmax and is
    # intentionally never applied.
    if flags not in _CACHE:
        _CACHE[flags] = _build(flags)
    nc = _CACHE[flags]

    w_bf = w_qkv.astype(ml_dtypes.bfloat16)
    in_maps = []
    for c in range(8):
        b = c // 4
        r = c % 4
        xrot = np.ascontiguousarray(
            np.concatenate([x[b, r * QR :], x[b, : r * QR]], axis=0)
        )
        in_maps.append(
            {
                "xb": xrot,
                "w_qkv": w_bf,
                "b_qkv": b_qkv,
                "g_pre": g_pre,
                "beta_pre": beta_pre,
                "g_post": g_post,
                "beta_post": beta_post,
            }
        )

    global _last_in_maps
    _last_in_maps = in_maps
    res = run_bass_kernel_spmd(nc, in_maps, core_ids=list(range(8)))
    out = np.empty((B, N, C), dtype=np.float32)
    for c in range(8):
        b = c // 4
        r = c % 4
        out[b, r * QR : (r + 1) * QR] = res.results[c]["out"]
    return out


# revision 10
# speedup vs baseline: 1.1495x; 1.0339x over previous
"""Fused pre-LN + QKV + attention + post-LN + residual kernel for TRN2.

Problem (nn_Attention_86517821210894):
    B=2, N=4096, C=512, H=8, D=64
    xn  = LN(x) ; qkv = xn @ w_qkv + b ; per-(b,h) softmax attention
    val = LN(attn_out) ; out = xn + val

Sharding (8 cores, zero collectives):
    core c -> batch b = c // 4, query-row block r = c % 4 (1024 rows).
    Each core receives x[b] ROTATED so its query block is rows 0:1024
    (softmax and the value sum are permutation-invariant over keys, so
    rotating the key order changes nothing), builds K/V for all 4096
    keys, and produces out[b, r*1024:(r+1)*1024].  The K/V qkv matmul is
    recomputed by the 4 cores of a batch; this trades ~30% extra PE work
    for zero inter-core communication.

Device pipeline per core (PE-throughput oriented: the Tensor engine is
the pacing resource at ~300us of matmul streaming; ScalarE's exp stream
is ~260us; the emission order keeps both fed continuously and avoids
HAM clock-down windows):
    1. warmup matmul burst on a zeroed dummy (no data deps) so the PE
       HAM reaches K=8/8 before real work, and stays warm through the
       LN prefix
    2. pre-LN: bn_stats/bn_aggr on VectorE, batched rsqrt via the exp
       table, normalize straight to bf16 (the bf16 rows double as the
       residual source), PE-transpose into xnT [C, N] bf16
    3. qkv from xnT: v [N, 8, 65] (ones column -> AV also computes the
       softmax denominator), kT per head pair, qT for the query block
    4. scoresT[k, q] = kT-slice.T @ qT-slice, two heads packed in the
       PE array (row groups 0-63/64-127 run concurrently), grouped 3
       slots per PSUM tile so each ScalarE exp covers 1536 elem/part.  Max-subtraction is skipped:
       scores*0.125 has |z| < ~2 for LN'd inputs so exp is in range;
       softmax is shift-invariant so the result matches the reference.
    5. AV: valT_aug[65, q] accumulated over key chunks in PSUM
    6. epilogue per (pair, qb) is split and pipelined into the NEXT
       stream: pav->SBUF copies (frees PSUM) at the stream boundary,
       bf16 PE transposes + per-head divide a few chunks later, and the
       post-LN + residual + DMA chain one segment after that.
"""

import sys

sys.path.insert(0, "/opt/trn_rl_repo")

import numpy as np

B, N, C, H = 2, 4096, 512, 8
D = C // H
QR = N // 4  # query rows per core
EPS = 1e-5
SCALE = float(D) ** -0.5

_CACHE = {}


def _build(flags):
    (use_g_pre, use_beta_pre, use_g_post, use_beta_post, use_b_q, use_b_v) = flags

    import concourse.bacc as bacc
    import concourse.bass as bass
    import concourse.tile as tile
    from concourse import mybir
    from concourse.masks import make_identity

    f32 = mybir.dt.float32
    bf16 = mybir.dt.bfloat16
    AF = mybir.ActivationFunctionType
    ALU = mybir.AluOpType

    nc = bacc.Bacc(
        "TRN2", target_bir_lowering=False, debug=False, enable_asserts=False
    )

    xb = nc.dram_tensor("xb", [N, C], f32, kind="ExternalInput").ap()
    w = nc.dram_tensor("w_qkv", [C, 3 * C], bf16, kind="ExternalInput").ap()
    bqkv = nc.dram_tensor("b_qkv", [3 * C], f32, kind="ExternalInput").ap()
    g_pre = nc.dram_tensor("g_pre", [C], f32, kind="ExternalInput").ap()
    beta_pre = nc.dram_tensor("beta_pre", [C], f32, kind="ExternalInput").ap()
    g_post = nc.dram_tensor("g_post", [C], f32, kind="ExternalInput").ap()
    beta_post = nc.dram_tensor("beta_post", [C], f32, kind="ExternalInput").ap()
    out = nc.dram_tensor("out", [QR, C], f32, kind="ExternalOutput").ap()

    NT = N // 128  # 32 row tiles of x[b]
    QT = QR // 128  # 8 row tiles of the query block
    CCH = C // 128  # 4 contraction chunks
    KC = N // 128  # 32 key chunks
    NPAIR = H // 2
    NQB = QR // 512  # 2 query blocks of 512
    GS = 3  # score slots per exp group

    def bcast(vec_ap, p):
        return bass.AP(
            tensor=vec_ap.tensor, offset=vec_ap.offset, ap=[[0, p], *vec_ap.ap]
        )

    with tile.TileContext(nc) as tc:
        with (
            tc.tile_pool(name="consts", bufs=1) as consts,
            tc.tile_pool(name="ln_in", bufs=4) as ln_in,
            tc.tile_pool(name="stats", bufs=8) as stats,
            tc.tile_pool(name="resid", bufs=1) as res_pool,
            tc.tile_pool(name="xnT", bufs=1) as xnT_pool,
            tc.tile_pool(name="vsb", bufs=1) as v_pool,
            tc.tile_pool(name="kT", bufs=4) as kT_pool,
            tc.tile_pool(name="qT", bufs=4) as qT_pool,
            tc.tile_pool(name="expT", bufs=3) as expT_pool,
            tc.tile_pool(name="valT", bufs=4) as valT_pool,
            tc.tile_pool(name="valasm", bufs=1) as val_pool,
            tc.tile_pool(name="outp", bufs=1) as out_pool,
            tc.tile_pool(name="ps3", bufs=2, space="PSUM") as ps3,
            tc.tile_pool(name="pav", bufs=2, space="PSUM") as psum_av,
        ):
            # ---- warmup burst: bring the PE HAM to K=8/8 immediately and
            # keep it warm through the LN prefix (no data dependencies) ----
            dummy = consts.tile([128, 512], bf16)
            nc.vector.memset(dummy, 0.0)
            pw = ps3.tile([128, 3, 512], f32, tag="ps3")
            for _ in range(26):
                nc.tensor.matmul(pw[:, 0, :], dummy[:, 0:128], dummy)
            del pw

            seed_b = consts.tile([128, 1], f32)
            nc.vector.memset(seed_b, 0.5 * 0.6931471805599453 * 127.0)
            # trigger the exp ACT_TABLE_LOAD right away (one-time ~2.7us)
            tbl_warm = stats.tile([128, 1], f32, tag="tblw")
            nc.scalar.activation(out=tbl_warm, in_=seed_b, func=AF.Exp, scale=0.01)

            # ---- x row tiles: first DMAs in the queue so LN starts early ----
            xt_tiles = {}

            def fetch(i):
                xt = ln_in.tile([128, C], f32, tag="xt", name=f"xt{i}")
                nc.sync.dma_start(out=xt, in_=xb[i * 128 : (i + 1) * 128, :])
                xt_tiles[i] = xt

            for i in range(4):
                fetch(i)

            # ---- constants ----
            w_sb = consts.tile([128, CCH, 3 * C], bf16)
            nc.sync.dma_start(out=w_sb, in_=w.rearrange("(cc p) m -> p cc m", p=128))

            g_pre_t = beta_pre_t = g_post_t = beta_post_t = None
            if use_g_pre:
                g_pre_t = consts.tile([128, C], f32)
                nc.sync.dma_start(out=g_pre_t, in_=bcast(g_pre, 128))
            if use_beta_pre:
                beta_pre_t = consts.tile([128, C], f32)
                nc.sync.dma_start(out=beta_pre_t, in_=bcast(beta_pre, 128))
            if use_g_post:
                g_post_t = consts.tile([128, C], f32)
                nc.sync.dma_start(out=g_post_t, in_=bcast(g_post, 128))
            if use_beta_post:
                beta_post_t = consts.tile([128, C], f32)
                nc.sync.dma_start(out=beta_post_t, in_=bcast(beta_post, 128))
            bq_t = None
            if use_b_q:
                bq_t = consts.tile([128, CCH, 1], f32)
                nc.sync.dma_start(
                    out=bq_t, in_=bqkv[0:C].rearrange("(cc p) -> p cc 1", p=128)
                )
            bv_t = None
            if use_b_v:
                bv_t = consts.tile([128, C], f32)
                nc.sync.dma_start(out=bv_t, in_=bcast(bqkv[2 * C : 3 * C], 128))

            # ---- persistent tensors ----
            res = res_pool.tile([128, QT, C], bf16)  # bf16 xn rows: residual src
            xnT = xnT_pool.tile([128, CCH, N], bf16)
            v_sb = v_pool.tile([128, KC, H, D + 1], bf16)
            # cols 0:64 = head values, col 64 = softmax denominator,
            # cols 65:96 = xbar-transpose padding (never read)
            val_asm = val_pool.tile([128, QT, H, 96], bf16)

            nc.vector.memset(v_sb[:, :, :, D : D + 1], 1.0)

            def rsqrt_into(dst, a4, w, tag):
                """dst = 1/sqrt(a4), a4 > 0, [128, w] f32.

                Bit-trick log2 seed evaluated through ScalarE Exp (the
                resident ACT table) + 2 Newton iterations on VectorE."""
                ai = a4.bitcast(mybir.dt.int32)
                fi = stats.tile([128, w], f32, tag=tag + "_f")
                nc.vector.tensor_copy(out=fi, in_=ai)
                nc.scalar.activation(
                    out=dst,
                    in_=fi,
                    func=AF.Exp,
                    scale=-0.5 * 0.6931471805599453 / 8388608.0,
                    bias=seed_b,
                )
                for _ in range(2):
                    t = stats.tile([128, w], f32, tag=tag + "_t")
                    nc.vector.tensor_mul(out=t, in0=dst, in1=dst)
                    nc.vector.tensor_mul(out=t, in0=t, in1=a4)
                    nc.vector.tensor_scalar(
                        out=t,
                        in0=t,
                        scalar1=-0.5,
                        scalar2=1.5,
                        op0=ALU.mult,
                        op1=ALU.add,
                    )
                    nc.vector.tensor_mul(out=dst, in0=dst, in1=t)

            def transpose_into(dstT, src, col0):
                # xbar DMA transpose: dst[p, cc, n] = src[n, cc*128+p]
                nc.sync.dma_start(
                    out=dstT[:, :, col0 : col0 + 128], in_=src, transpose=True
                )

            def produce_v(kc):
                pv = ps3.tile([128, 512], f32, tag="ps3")
                for cc in range(CCH):
                    nc.tensor.matmul(
                        pv,
                        xnT[:, cc, kc * 128 : (kc + 1) * 128],
                        w_sb[:, cc, 2 * C : 3 * C],
                        start=(cc == 0),
                        stop=(cc == CCH - 1),
                    )
                src = pv.rearrange("p (h d) -> p h d", d=D)
                dst = v_sb[:, kc, :, 0:D]
                if use_b_v:
                    nc.vector.tensor_add(
                        out=dst, in0=src, in1=bv_t.rearrange("p (h d) -> p h d", d=D)
                    )
                else:
                    nc.scalar.copy(out=dst, in_=src)

            def produce_kT(pair, kT, rc, eng=None):
                pk = ps3.tile([128, 512], f32, tag="ps3")
                for cc in range(CCH):
                    nc.tensor.matmul(
                        pk,
                        w_sb[:, cc, C + pair * 128 : C + (pair + 1) * 128],
                        xnT[:, cc, rc * 512 : (rc + 1) * 512],
                        start=(cc == 0),
                        stop=(cc == CCH - 1),
                    )
                (eng or nc.vector).tensor_copy(
                    out=kT[:, rc * 512 : (rc + 1) * 512], in_=pk
                )

            def produce_qT(pair, qT, rc, eng=None):
                pq = ps3.tile([128, 512], f32, tag="ps3")
                for cc in range(CCH):
                    nc.tensor.matmul(
                        pq,
                        w_sb[:, cc, pair * 128 : (pair + 1) * 128],
                        xnT[:, cc, rc * 512 : (rc + 1) * 512],
                        start=(cc == 0),
                        stop=(cc == CCH - 1),
                    )
                if use_b_q:
                    (eng or nc.vector).tensor_scalar_add(
                        out=qT[:, rc * 512 : (rc + 1) * 512],
                        in0=pq,
                        scalar1=bq_t[:, pair, :],
                    )
                else:
                    (eng or nc.vector).tensor_copy(
                        out=qT[:, rc * 512 : (rc + 1) * 512], in_=pq
                    )

            # ---- attention slice machinery ----
            class AttState:
                def __init__(self):
                    self.group = None
                    self.pos = 0
                    self.pending = []
                    self.exp_of = {}
                    self.pavs = None
                    self.av_next = 0
                    self.vts = None

            def flush(st):
                ex = expT_pool.tile([128, GS, 512], bf16, tag="expT")
                nc.scalar.activation(
                    out=ex[:, 0 : st.pos, :],
                    in_=st.group[:, 0 : st.pos, :],
                    func=AF.Exp,
                    scale=SCALE,
                )
                for key, p in st.pending:
                    st.exp_of[key] = (ex, p)
                st.pending.clear()
                st.group = None
                st.pos = 0

            def av_drain(st, pair):
                if st.pavs is None:
                    pav_lo = psum_av.tile([128, 512], f32, tag="pav")
                    pav_hi = psum_av.tile([128, 512], f32, tag="pav")
                    st.pavs = (pav_lo, pav_hi)
                while st.av_next < KC and (st.av_next, 1) in st.exp_of:
                    kc = st.av_next
                    for h_idx in range(2):
                        ex, p = st.exp_of.pop((kc, h_idx))
                        nc.tensor.matmul(
                            st.pavs[h_idx][0 : D + 1, :],
                            v_sb[:, kc, 2 * pair + h_idx, :],
                            ex[:, p, :],
                            start=(kc == 0),
                            stop=(kc == KC - 1),
                        )
                    st.av_next += 1

            def scores_chunk(st, pair, qb, kcs, kT, qT, flush_end=True):
                for kc in kcs:
                    for h_idx in range(2):
                        if st.group is None:
                            st.group = ps3.tile([128, GS, 512], f32, tag="ps3")
                            st.pos = 0
                        base = h_idx * 64
                        nc.tensor.matmul(
                            st.group[:, st.pos, :],
                            kT[base : base + 64, kc * 128 : (kc + 1) * 128],
                            qT[base : base + 64, qb * 512 : (qb + 1) * 512],
                        )
                        st.pending.append(((kc, h_idx), st.pos))
                        st.pos += 1
                        if st.pos == GS:
                            # drain AV for exps from EARLIER flushes first so
                            # the PE never waits on the exp it just requested
                            av_drain(st, pair)
                            flush(st)
                if flush_end and st.group is not None:
                    av_drain(st, pair)
                    flush(st)

            def stream_close(st, pair):
                # close the open score group at end-of-stream so later ps3
                # allocations can never wedge behind an unflushed group
                av_drain(st, pair)
                if st.group is not None:
                    flush(st)

            def av_copy_out(st, pair):
                """End of a (pair, qb) stream: last flush + AV matmuls, then
                move the PSUM accumulators to SBUF so the pav banks free up
                for the next stream."""
                av_drain(st, pair)
                if st.group is not None:
                    flush(st)
                av_drain(st, pair)
                assert st.av_next == KC
                vts = []
                for h_idx in range(2):
                    vt = valT_pool.tile([96, 512], bf16, tag="valT")
                    nc.vector.tensor_copy(
                        out=vt[0 : D + 1, :], in_=st.pavs[h_idx][0 : D + 1, :]
                    )
                    vts.append(vt)
                st.pavs = None
                st.vts = vts

            def av_transpose(st, pair, qb, h_idx):
                """Deferred epilogue: xbar DMA transpose of one valT tile
                into val_asm row-major, plus a scatter DMA that fans the
                denominator row out across partitions."""
                h = 2 * pair + h_idx
                nc.sync.dma_start(
                    out=val_asm[:, qb * 4 : (qb + 1) * 4, h, :],
                    in_=st.vts[h_idx],
                    transpose=True,
                )

            # ---- phase 5, split ----
            ot_tiles = {}

            def phase5_div(pair, qb):
                for qtile in range(qb * 4, qb * 4 + 4):
                    if qtile not in ot_tiles:
                        ot_tiles[qtile] = out_pool.tile(
                            [128, C], f32, tag=f"ot{qtile}", name=f"ot{qtile}"
                        )
                    ot = ot_tiles[qtile]
                    va = val_asm[:, qtile]
                    rs = stats.tile([128, 2], f32, tag="rs")
                    nc.vector.reciprocal(
                        out=rs, in_=va[:, 2 * pair : 2 * pair + 2, D : D + 1]
                    )
                    for h_idx in range(2):
                        h = 2 * pair + h_idx
                        nc.vector.tensor_scalar_mul(
                            out=ot[:, h * D : (h + 1) * D],
                            in0=va[:, h, 0:D],
                            scalar1=rs[:, h_idx : h_idx + 1],
                        )

            def phase5_ln(qtiles):
                gw = len(qtiles)
                m4 = stats.tile([128, gw], f32, tag=f"m4b{gw}")
                a4b = stats.tile([128, gw], f32, tag=f"a4b{gw}")
                r4 = stats.tile([128, gw], f32, tag=f"r4b{gw}")
                for jj, qtile in enumerate(qtiles):
                    ot = ot_tiles[qtile]
                    if use_b_v:
                        nc.vector.tensor_add(out=ot, in0=ot, in1=bv_t)
                    st6 = stats.tile([128, 6], f32, tag="bn6")
                    nc.vector.bn_stats(out=st6, in_=ot)
                    mv = stats.tile([128, 2], f32, tag="mv")
                    nc.vector.bn_aggr(out=mv, in_=st6)
                    nc.vector.tensor_copy(out=m4[:, jj : jj + 1], in_=mv[:, 0:1])
                    nc.vector.tensor_copy(out=a4b[:, jj : jj + 1], in_=mv[:, 1:2])
                nc.vector.tensor_scalar_add(out=a4b, in0=a4b, scalar1=EPS)
                rsqrt_into(r4, a4b, gw, f"p5{gw}")
                for jj, qtile in enumerate(qtiles):
                    ot = ot_tiles[qtile]
                    nc.vector.tensor_scalar(
                        out=ot,
                        in0=ot,
                        scalar1=m4[:, jj : jj + 1],
                        scalar2=r4[:, jj : jj + 1],
                        op0=ALU.subtract,
                        op1=ALU.mult,
                    )
                    if use_g_post:
                        nc.vector.tensor_mul(out=ot, in0=ot, in1=g_post_t)
                    if use_beta_post:
                        nc.vector.tensor_add(out=ot, in0=ot, in1=beta_post_t)
                    nc.vector.tensor_add(out=ot, in0=ot, in1=res[:, qtile, :])
                    nc.sync.dma_start(
                        out=out[qtile * 128 : (qtile + 1) * 128, :], in_=ot
                    )

            # ================= emission =================
            kTs = [
                kT_pool.tile([128, N], bf16, tag="kT", name=f"kT{p}")
                for p in range(NPAIR)
            ]
            qTs = [
                qT_pool.tile([128, QR], bf16, tag="qT", name=f"qT{p}")
                for p in range(NPAIR)
            ]
            states = [[AttState() for _ in range(NQB)] for _ in range(NPAIR)]

            # phase 1+2 fused; first two LN groups are 2 tiles for latency,
            # pair-0 qb-0 scores+exp+AV trickle one column-group behind
            st00 = states[0][0]
            groups = [[0, 1], [2, 3]] + [
                list(range(4 * g, 4 * g + 4)) for g in range(1, NT // 4)
            ]
            rc_done = 0
            for grp in groups:
                gw = len(grp)
                xts, mvs = [], []
                for i in grp:
                    xt = xt_tiles.pop(i)
                    if i + 4 < NT:
                        fetch(i + 4)
                    st6 = stats.tile([128, 6], f32, tag="bn6")
                    nc.vector.bn_stats(out=st6, in_=xt)
                    mv = stats.tile([128, 2], f32, tag="mv")
                    nc.vector.bn_aggr(out=mv, in_=st6)
                    xts.append(xt)
                    mvs.append(mv)
                a4 = stats.tile([128, gw], f32, tag=f"a4_{gw}")
                for j in range(gw):
                    nc.vector.tensor_copy(out=a4[:, j : j + 1], in_=mvs[j][:, 1:2])
                nc.vector.tensor_scalar_add(out=a4, in0=a4, scalar1=EPS)
                r4 = stats.tile([128, gw], f32, tag=f"r4_{gw}")
                rsqrt_into(r4, a4, gw, f"p1_{gw}")
                for j, i in enumerate(grp):
                    if i < QT:
                        dst = res[:, i, :]
                    else:
                        dst = ln_in.tile([128, C], bf16, tag="xbf")
                    nc.vector.tensor_scalar(
                        out=dst,
                        in0=xts[j],
                        scalar1=mvs[j][:, 0:1],
                        scalar2=r4[:, j : j + 1],
                        op0=ALU.subtract,
                        op1=ALU.mult,
                    )
                    if use_g_pre:
                        nc.vector.tensor_mul(out=dst, in0=dst, in1=g_pre_t)
                    if use_beta_pre:
                        nc.vector.tensor_add(out=dst, in0=dst, in1=beta_pre_t)
                    transpose_into(xnT, dst, i * 128)
                    produce_v(i)
                completed = grp[-1] + 1
                while completed >= 4 * (rc_done + 1):
                    rc = rc_done
                    produce_kT(0, kTs[0], rc)
                    if rc < NQB:
                        produce_qT(0, qTs[0], rc)
                    scores_chunk(
                        st00, 0, 0, range(4 * rc, 4 * rc + 4), kTs[0], qTs[0]
                    )
                    # produce pair 1's first kT blocks inside the trickle tail
                    if rc >= 4:
                        produce_kT(1, kTs[1], rc - 4, eng=nc.vector)
                    rc_done += 1


            # ---- pipelined (pair, qb) streams ----
            # stream order after (0,0): (1,0) (2,0) (3,0) (0,1) (1,1) (2,1) (3,1)
            # av_copy_out of the previous stream is deferred until after the
            # next stream's first chunk; transposes / divides / post-LN land
            # progressively further in so no engine drains at a boundary.
            order = [(0, 1)] + [
                (p, qb) for p in range(1, NPAIR) for qb in (0, 1)
            ]
            prev = (0, 0)
            for pair, qb in order:
                st = states[pair][qb]
                pst = states[prev[0]][prev[1]]
                for ci in range(11):  # chunks of 3 kc (exactly 2 exp groups)
                    kcs = range(3 * ci, min(3 * ci + 3, KC))
                    scores_chunk(
                        st, pair, qb, kcs, kTs[pair], qTs[pair], flush_end=False
                    )
                    if ci == 0:
                        av_copy_out(pst, prev[0])
                    elif ci == 1:
                        av_transpose(pst, prev[0], prev[1], 0)
                        av_transpose(pst, prev[0], prev[1], 1)
                        pst.vts = None
                    elif ci == 2:
                        phase5_div(prev[0], prev[1])
                    elif ci == 3 and (pair, qb) == (3, 1):
                        phase5_ln([0, 1, 2, 3])
                    # kT/qT production for the pair after this one, spread so
                    # every stream stays under the exp-stream pace
                    if pair + 1 < NPAIR:
                        nxt = pair + 1
                        if qb == 0 and pair >= 1:
                            if 2 <= ci <= 6:
                                produce_kT(nxt, kTs[nxt], ci - 2, eng=nc.vector)
                        elif qb == 1 and pair >= 1:
                            if 1 <= ci <= 3:
                                produce_kT(nxt, kTs[nxt], ci + 4, eng=nc.vector)
                            elif ci == 4:
                                produce_qT(nxt, qTs[nxt], 0, eng=nc.vector)
                            elif ci == 5:
                                produce_qT(nxt, qTs[nxt], 1, eng=nc.vector)
                        elif qb == 1 and pair == 0:
                            if 1 <= ci <= 4:
                                produce_kT(nxt, kTs[nxt], ci + 3, eng=nc.vector)
                            elif ci == 5:
                                produce_qT(nxt, qTs[nxt], 0, eng=nc.vector)
                            elif ci == 6:
                                produce_qT(nxt, qTs[nxt], 1, eng=nc.vector)
                stream_close(st, pair)
                prev = (pair, qb)

            st31 = states[3][1]
            av_copy_out(st31, 3)
            av_transpose(st31, 3, 1, 0)
            av_transpose(st31, 3, 1, 1)
            st31.vts = None
            phase5_div(3, 1)
            phase5_ln([4, 5, 6, 7])

    nc.compile()
    return nc


def kernel(x, w_qkv, b_qkv, g_pre, beta_pre, g_post, beta_post):
    import ml_dtypes
    from concourse.bass_utils import run_bass_kernel_spmd

    x = np.asarray(x, dtype=np.float32)
    w_qkv = np.asarray(w_qkv, dtype=np.float32)
    b_qkv = np.asarray(b_qkv, dtype=np.float32)
    g_pre = np.asarray(g_pre, dtype=np.float32)
    beta_pre = np.asarray(beta_pre, dtype=np.float32)
    g_post = np.asarray(g_post, dtype=np.float32)
    beta_post = np.asarray(beta_post, dtype=np.float32)

    flags = (
        not np.all(g_pre == 1.0),
        not np.all(beta_pre == 0.0),
        not np.all(g_post == 1.0),
        not np.all(beta_post == 0.0),
        not np.all(b_qkv[0:C] == 0.0),
        not np.all(b_qkv[2 * C : 3 * C] == 0.0),
    )
    # NOTE: b_qkv[C:2C] (the K bias) provably cancels in softmax and is
    # intentionally never applied.
    if flags not in _CACHE:
        _CACHE[flags] = _build(flags)
    nc = _CACHE[flags]

    w_bf = w_qkv.astype(ml_dtypes.bfloat16)
    in_maps = []
    for c in range(8):
        b = c // 4
        r = c % 4
        xrot = np.ascontiguousarray(
            np.concatenate([x[b, r * QR :], x[b, : r * QR]], axis=0)
        )
        in_maps.append(
            {
                "xb": xrot,
                "w_qkv": w_bf,
                "b_qkv": b_qkv,
                "g_pre": g_pre,
                "beta_pre": beta_pre,
                "g_post": g_post,
                "beta_post": beta_post,
            }
        )

    global _last_in_maps
    _last_in_maps = in_maps
    res = run_bass_kernel_spmd(nc, in_maps, core_ids=list(range(8)))
    out = np.empty((B, N, C), dtype=np.float32)
    for c in range(8):
        b = c // 4
        r = c % 4
        out[b, r * QR : (r + 1) * QR] = res.results[c]["out"]
    return out
